# revision 66
# baseline (speedup 1.0000x reference)
"""Causal multi-head attention Trainium2 kernel (8 NeuronCores).

Problem: B=4, L=2048, D=1024, 16 heads x (dh=64, dv=64), causal mask.
Sharding: data-parallel over batch (4) x tensor-parallel over heads (2 groups
of 8). Core c handles batch c//2, head-group c%2. Each core computes its
partial output projection (ctx_g @ Wo_g); the host sums the two head-group
partials per batch and adds the bias.

v2: software-pipelined. The prologue transposes x (PE) into a resident
xT [d, l] tile, computes V = x@Wv (+ones column for the fused softmax
denominator) and Q^T/K^T for head-pair 0. The main loop runs flash-style
attention per head-pair while dribbling the next head-pair's Q^T/K^T
projection matmuls between attention groups — keeping the PE dense so the
HAM clock gate stays at 2.4 GHz.
S^T tiles = K@Q^T; exp on ACT (scale=1/8 folded in); causal diagonal via one
tril tensor_mul per diag k-tile + width-restricted PV; ones column of V_aug
gives the softmax denominator in PSUM row 64; normalize with DVE
reciprocal_approx_fast + gpsimd partition_broadcast + DVE multiply into the
resident ctx^T tile; output projection at the end.

v3: bf16 datapath (tolerance is 2e-2; halves DMA bytes, LDWEIGHTS stalls,
SBUF traffic; PE matmul rate is 1 cycle/row either way). Causal-restricted
S^T matmul + exp widths on diagonal k-tiles. Wo prefetched right after the
prologue.

v4 (final, ~332us vs 423us baseline): space heater removed (the pipeline
is dense enough that DVFS stays up on real work). Output projection
dribbled into hp3's attention via OutEmitter, leaving only j=3's four
l-tiles for the drain. hp0 runs its q-chunks descending so j=3's 32 S
matmuls hide the cold-ACT exp latency at the prologue boundary. Normalize
copies ctx out of PSUM immediately (pctx bank freed after ~1us instead of
the full recip/broadcast chain, unblocking the next chunk's first PV).
DMA traffic split across queues: bulk weight loads on the ACT hwdge
queue, x slices alternating sync/ACT, ost stores via gpsimd software DGE
- the gpsimd partition_broadcast in the softmax normalize rides sync-queue
DIRECT2D slots and must not sit behind bulk transfers.
"""

import numpy as np
from contextlib import ExitStack

import concourse.bass as bass
import concourse.tile as tile
from concourse import bacc, mybir
from concourse.masks import make_identity

F32 = mybir.dt.float32
F32R = mybir.dt.float32r
BF16 = mybir.dt.bfloat16
AF = mybir.ActivationFunctionType

B, L, D = 4, 2048, 1024
N_HEAD, DH, DV = 16, 64, 64
N_CORES = 8
HPC = N_HEAD // 2          # heads per core (8)
OC = HPC * DH              # per-core projection width (512)
NHP = HPC // 2             # head-pairs per core (4)


class ProjEmitter:
    """Q^T/K^T projection for one head-pair, emitted in per-(proj,chunk)
    units so the matmuls interleave with attention of the previous pair."""

    def __init__(self, nc, hp, pools, xt, wq, wk, nch):
        self.nc = nc
        self.xt = xt
        qkp, wp, self.psP = pools
        self.wq_sb = wp.tile([128, 8, 128], BF16, tag="wq")
        self.wk_sb = wp.tile([128, 8, 128], BF16, tag="wk")
        # ACT hwdge queue: keeps bulk weight loads off the sync queue,
        # whose DIRECT2D slots the normalize partition_broadcasts ride
        nc.scalar.dma_start(
            out=self.wq_sb,
            in_=wq[:, hp * 128:(hp + 1) * 128].rearrange("(t p) o -> p t o", p=128))
        nc.scalar.dma_start(
            out=self.wk_sb,
            in_=wk[:, hp * 128:(hp + 1) * 128].rearrange("(t p) o -> p t o", p=128))
        self.qt = qkp.tile([128, nch * 512], BF16, tag="qt")
        self.kt = qkp.tile([128, nch * 512], BF16, tag="kt")
        self.units = [(w, d, c) for w, d in ((self.wq_sb, self.qt),
                                             (self.wk_sb, self.kt))
                      for c in range(nch)]
        self.i = 0

    def step(self):
        if self.i >= len(self.units):
            return False
        w_sb, dst, c = self.units[self.i]
        self.i += 1
        nc = self.nc
        pp = self.psP.tile([128, 512], F32, tag="pp")
        for d in range(8):
            nc.tensor.matmul(pp, w_sb[:, d, :],
                             self.xt[:, d, c * 512:(c + 1) * 512],
                             start=(d == 0), stop=(d == 7))
        nc.vector.tensor_copy(dst[:, c * 512:(c + 1) * 512], pp)
        return True

    def drain(self):
        while self.step():
            pass


class OutEmitter:
    """Output projection, dribbled into hp3's attention: after head-pair 3
    finishes q-chunk j, the four l-tiles 4j..4j+3 have all heads' ctx ready
    and their out-projection can interleave with the remaining attention."""

    def __init__(self, nc, pools, ct, wo_sb, out):
        self.nc = nc
        self.phco, self.psP = pools
        self.ct, self.wo_sb, self.out = ct, wo_sb, out
        self.queue = []
        self.ost = {}
        self.pa = {}

    def add_lt(self, lt):
        self.queue.extend([("full", lt, 0), ("full", lt, 1)])

    def add_partial(self, lt):
        # v=0..2 contraction only: these l-tiles' head-pair 0-2 ctx is
        # ready before hp3's attention even starts, so 3/4 of the last
        # tiles' projection work dribbles instead of sitting in the drain
        self.queue.extend([("pA", lt, 0), ("pA", lt, 1)])

    def add_finish(self, lt):
        self.queue.extend([("fin", lt, 0), ("fin", lt, 1)])

    def step(self):
        if not self.queue:
            return False
        kind, lt, n = self.queue.pop(0)
        nc = self.nc
        if kind == "pA":
            pp = self.psP.tile([128, 512], F32, tag="pp", name="ppo")
            for v in range(3):
                nc.tensor.matmul(pp, self.ct[:, v, lt * 128:(lt + 1) * 128],
                                 self.wo_sb[:, v, n * 512:(n + 1) * 512],
                                 start=(v == 0), stop=(v == 2))
            pa = self.phco.tile([128, 512], F32, tag="pA", bufs=8,
                                name=f"pA{lt}_{n}")
            nc.vector.tensor_copy(pa, pp)
            self.pa[(lt, n)] = pa
            return True
        if n == 0:
            self.ost[lt] = self.phco.tile([128, D], F32, tag="ost",
                                          name=f"ost{lt}")
        ost = self.ost[lt]
        pp = self.psP.tile([128, 512], F32, tag="pp", name="ppo")
        if kind == "fin":
            nc.tensor.matmul(pp, self.ct[:, 3, lt * 128:(lt + 1) * 128],
                             self.wo_sb[:, 3, n * 512:(n + 1) * 512],
                             start=True, stop=True)
            nc.vector.tensor_add(ost[:, n * 512:(n + 1) * 512], pp,
                                 self.pa.pop((lt, n)))
        else:
            for v in range(4):
                nc.tensor.matmul(pp, self.ct[:, v, lt * 128:(lt + 1) * 128],
                                 self.wo_sb[:, v, n * 512:(n + 1) * 512],
                                 start=(v == 0), stop=(v == 3))
            nc.vector.tensor_copy(ost[:, n * 512:(n + 1) * 512], pp)
        if n == 1:
            # gpsimd software DGE: ost stores would otherwise queue ahead
            # of hp3's normalize broadcasts on the sync queue
            nc.gpsimd.dma_start(out=self.out[lt * 128:(lt + 1) * 128, :],
                                in_=ost)
            del self.ost[lt]
        return True

    def drain(self):
        while self.step():
            pass


def build_nc(l=L):
    assert l % 512 == 0
    nch = l // 512           # q-chunks
    nlt = l // 128           # l-tiles
    nc = bacc.Bacc("TRN2", target_bir_lowering=False, debug=False,
                   num_devices=N_CORES)

    x = nc.dram_tensor("x", [l, D], BF16, kind="ExternalInput").ap()
    wq = nc.dram_tensor("wq", [D, OC], BF16, kind="ExternalInput").ap()
    wk = nc.dram_tensor("wk", [D, OC], BF16, kind="ExternalInput").ap()
    wv = nc.dram_tensor("wv", [D, OC], BF16, kind="ExternalInput").ap()
    wo = nc.dram_tensor("wo", [OC, D], BF16, kind="ExternalInput").ap()
    out = nc.dram_tensor("out", [l, D], F32, kind="ExternalOutput").ap()

    with tile.TileContext(nc) as tc, ExitStack() as ctx:
        top = ctx.enter_context(tc.tile_pool(name="top", bufs=1))
        xtp = ctx.enter_context(tc.tile_pool(name="xtp", bufs=1))
        qkp = ctx.enter_context(tc.tile_pool(name="qkp", bufs=2))
        wp = ctx.enter_context(tc.tile_pool(name="wp", bufs=2))
        phco = ctx.enter_context(tc.tile_pool(name="phco", bufs=3))

        # V: [128(l), ltile, head, 65] - col 64 is ones (softmax denominator)
        vt = top.tile([128, nlt, HPC, DH + 1], BF16)
        ct = top.tile([128, NHP, l], BF16)        # normalized ctx^T
        tril = top.tile([128, 128], BF16)
        ones = top.tile([128, 1], F32)
        warm = top.tile([128, 1], BF16)
        xt = xtp.tile([128, 8, l], BF16)          # x^T, d-major

        nc.vector.memset(ones, 1.0)
        # warm-up exp: loads the ACT function table during the DMA-bound
        # startup instead of stalling the first attention group
        nc.scalar.activation(warm, ones, AF.Exp, scale=0.125)
        nc.vector.tensor_copy(
            vt[:, :, :, DV:DV + 1].rearrange("p t h c -> p (t h) c"),
            ones.broadcast_to((128, nlt * HPC, 1)))
        # causal keep-mask for S^T diag blocks: tril[k, q] = 1.0 iff q >= k
        nc.gpsimd.memset(tril, 0.0)
        nc.gpsimd.affine_select(
            out=tril, in_=tril, compare_op=mybir.AluOpType.is_gt,
            fill=1.0, base=0, pattern=[[-1, 128]], channel_multiplier=1)

        # ---------------- Prologue: transpose + V + QK(hp=0) --------------
        with tc.tile_pool(name="pro", bufs=8) as pro, \
             tc.tile_pool(name="wvp", bufs=1) as wvp, \
             tc.tile_pool(name="psPro", bufs=2, space="PSUM") as psPro, \
             tc.tile_pool(name="psT", bufs=3, space="PSUM") as psT:
            ident = wvp.tile([128, 128], BF16)
            make_identity(nc, ident)
            # chunk-0 x slices first so the first transposes start ASAP;
            # only then the Wv load (needed one chunk later)
            # x slices alternate across both hwdge queues
            xst0 = []
            for s in range(4):
                xst = pro.tile([128, D], BF16, tag="xst", name=f"xst0{s}")
                eng = nc.sync if s % 2 == 0 else nc.scalar
                eng.dma_start(out=xst, in_=x[s * 128:(s + 1) * 128, :])
                xst0.append(xst)
            # Wv via gpsimd software DGE: a third DMA channel, so both
            # hwdge queues stay exclusive to the x slices
            wv_sb = wvp.tile([128, 8, OC], BF16)
            nc.gpsimd.dma_start(out=wv_sb,
                                in_=wv.rearrange("(t p) o -> p t o", p=128))
            for c in range(nch):
                for s in range(4):
                    if c == 0:
                        xst = xst0[s]
                    else:
                        xst = pro.tile([128, D], BF16, tag="xst")
                        eng = nc.sync if s % 2 == 0 else nc.scalar
                        eng.dma_start(
                            out=xst,
                            in_=x[c * 512 + s * 128: c * 512 + (s + 1) * 128, :])
                    # all 8 d-blocks transpose into one 1-bank PSUM tile,
                    # drained by a single wide DVE copy (8x fewer copies)
                    pt8 = psT.tile([128, 8, 128], BF16, tag="pt")
                    for d in range(8):
                        nc.tensor.transpose(
                            pt8[:, d, :], xst[:, d * 128:(d + 1) * 128],
                            ident)
                    nc.vector.tensor_copy(
                        xt[:, :, c * 512 + s * 128: c * 512 + (s + 1) * 128],
                        pt8)
                # V for this l-chunk
                for m in range(4):
                    pp = psPro.tile([128, 512], F32, tag="pp")
                    for d in range(8):
                        nc.tensor.matmul(
                            pp, xt[:, d, (c * 4 + m) * 128:(c * 4 + m + 1) * 128],
                            wv_sb[:, d, :], start=(d == 0), stop=(d == 7))
                    nc.vector.tensor_copy(
                        vt[:, c * 4 + m, :, 0:DV],
                        pp.rearrange("p (h v) -> p h v", h=HPC))
                if c == 1:
                    # issue hp0's Wq/Wk DMAs mid-prologue so the QK drain
                    # at the end doesn't stall on them
                    em = ProjEmitter(nc, 0, (qkp, wp, psPro), xt, wq, wk, nch)
            em.drain()

        # Prefetch Wo now: the DMA queue is idle during attention, and the
        # output projection otherwise stalls ~14us on this load at the end.
        phc = ctx.enter_context(tc.tile_pool(name="phc", bufs=1))
        wo_sb = phc.tile([128, 4, D], BF16)
        nc.scalar.dma_start(out=wo_sb,
                            in_=wo.rearrange("(t p) o -> p t o", p=128))

        # ---------------- Main: attention + next-pair projections ---------
        with tc.tile_pool(name="phb", bufs=2) as phb, \
             tc.tile_pool(name="psS", bufs=2, space="PSUM") as psS, \
             tc.tile_pool(name="psPd", bufs=2, space="PSUM") as psPd, \
             tc.tile_pool(name="psC", bufs=2, space="PSUM") as psC:
            n_groups_hp = 2 * nch * (nch + 1)
            for hp in range(NHP):
                qt, kt = em.qt, em.kt
                if hp + 1 < NHP:
                    em = ProjEmitter(nc, hp + 1, (qkp, wp, psPd), xt, wq, wk,
                                     nch)
                    cadence = max(1, (n_groups_hp // 2) // (2 * nch))
                else:
                    em = OutEmitter(nc, (phco, psPd), ct, wo_sb, out)
                    cadence = 1
                gcount = 0

                def pv_step(g, j, pctx, pexp, po, H):
                    # masks + PV for group g (one group after its exp)
                    for r2 in range(2):
                        kt_i = 2 * g + r2
                        r = kt_i - 4 * j
                        c0 = 0
                        if r >= 0:      # diagonal k-tile
                            c0 = r * 128
                            nc.vector.tensor_mul(
                                pexp[:, r2, c0:c0 + 128],
                                pexp[:, r2, c0:c0 + 128], tril)
                        nc.tensor.matmul(
                            pctx[:, c0:512],
                            vt[:, kt_i, H, :],
                            pexp[:, r2, c0:512],
                            start=(kt_i == 0), stop=(kt_i == 4 * j + 3))

                # both heads interleaved at group level: two independent
                # dependency chains keep the PE busy through each other's
                # exp waits. hp0 runs j descending: at the prologue
                # boundary the ACT pipeline is cold, and j=3 offers 32 S
                # matmuls to hide the first exp latencies (j=0 only 8).
                jorder = range(nch - 1, -1, -1) if hp == 0 else range(nch)
                for j in jorder:
                    if hp == NHP - 1:
                        # final j: spread the remaining dribble units so
                        # some out-proj work still sits in the PE queue
                        # while the last normalize chain runs
                        cadence = 2 if j == nch - 1 else 1
                    n_g = 2 * (j + 1)
                    pctxs = {}
                    prevs = {0: None, 1: None}
                    for h in range(2):
                        pctxs[h] = psC.tile([DV + 1, 512], F32,
                                            tag="pctx", name=f"pctx{h}")
                    for g in range(n_g + 1):
                        for h in range(2):
                            po = 64 * h
                            H = 2 * hp + h
                            pexp = None
                            if g < n_g:
                                psc = psS.tile([128, 2, 512], F32,
                                               tag="psc", name=f"psc{h}")
                                for r2 in range(2):
                                    kt_i = 2 * g + r2
                                    # causal: diag k-tile kt_i only feeds
                                    # q columns >= (kt_i-4j)*128
                                    c0 = max(0, kt_i - 4 * j) * 128
                                    nc.tensor.matmul(
                                        psc[:, r2, c0:512],
                                        kt[po:po + DH,
                                           kt_i * 128:(kt_i + 1) * 128],
                                        qt[po:po + DH,
                                           j * 512 + c0:(j + 1) * 512],
                                        start=True, stop=True)
                                pexp = phb.tile([128, 2, 512], BF16,
                                                tag="pexp", bufs=10,
                                                name=f"pexp{h}")
                                cg = max(0, 2 * g - 4 * j) * 128
                                nc.scalar.activation(pexp[:, :, cg:512],
                                                     psc[:, :, cg:512],
                                                     AF.Exp, scale=0.125)
                            if prevs[h] is not None:
                                pv_step(prevs[h][0], j, pctxs[h],
                                        prevs[h][1], po, H)
                                gcount += 1
                                if em and cadence and gcount % cadence == 0:
                                    em.step()
                            prevs[h] = (g, pexp) if g < n_g else None
                    for h in range(2):
                        po = 64 * h
                        # free the pctx PSUM bank after two quick DVE copies
                        # (the next j's first PV waits on it); the
                        # recip/broadcast/scale dangle off the critical path.
                        # ctu is a base-0 staging tile so the scale-mul's two
                        # SBUF inputs share a base partition.
                        rs = phb.tile([1, 512], F32, tag="rs", name="rs")
                        nc.vector.tensor_copy(rs, pctxs[h][DV:DV + 1, :])
                        ctu = phb.tile([64, 512], BF16, tag="ctu",
                                       name=f"ctu{h}")
                        nc.vector.tensor_copy(ctu, pctxs[h][0:DV, :])
                        inv = phb.tile([1, 512], F32, tag="inv", name="inv")
                        nc.vector.reciprocal_approx_fast(out=inv, in_=rs)
                        bc = phb.tile([64, 512], F32, tag="bc", name="bc")
                        nc.gpsimd.partition_broadcast(out_ap=bc, in_ap=inv)
                        nc.vector.tensor_mul(
                            ct[po:po + DV, hp, j * 512:(j + 1) * 512],
                            ctu, bc)
                    if hp == NHP - 1:
                        # all heads' ctx for l-tiles 4j..4j+3 now ready;
                        # queue their output projection for dribbling
                        for lt in range(4 * j, 4 * j + 4):
                            em.add_lt(lt)
                em.drain()

    nc.compile()
    return nc


def _bf16(a):
    import ml_dtypes
    return np.ascontiguousarray(a).astype(ml_dtypes.bfloat16)


def make_in_maps(x, Wq, Wk, Wv, Wo):
    in_maps = []
    for c in range(N_CORES):
        b, g = c // 2, c % 2
        in_maps.append({
            "x": _bf16(x[b]),
            "wq": _bf16(Wq[:, g * OC:(g + 1) * OC]),
            "wk": _bf16(Wk[:, g * OC:(g + 1) * OC]),
            "wv": _bf16(Wv[:, g * OC:(g + 1) * OC]),
            "wo": _bf16(Wo[g * OC:(g + 1) * OC, :]),
        })
    return in_maps


_NC_CACHE = {}


def _get_nc():
    if "nc" not in _NC_CACHE:
        _NC_CACHE["nc"] = build_nc()
    return _NC_CACHE["nc"]


def _numpy_fallback(x, Wq, Wk, Wv, Wo, bo, mask):
    Bsz, Lq, _ = x.shape
    Q = (x @ Wq).reshape(Bsz, Lq, N_HEAD, DH).transpose(0, 2, 1, 3)
    K = (x @ Wk).reshape(Bsz, Lq, N_HEAD, DH).transpose(0, 2, 1, 3)
    V = (x @ Wv).reshape(Bsz, Lq, N_HEAD, DV).transpose(0, 2, 1, 3)
    s = np.einsum("bhqd,bhkd->bhqk", Q, K) / np.sqrt(np.float32(DH))
    s = np.where(mask, s, -np.inf)
    s = s - s.max(axis=-1, keepdims=True)
    p = np.exp(s)
    p /= p.sum(axis=-1, keepdims=True)
    ctxv = np.einsum("bhqk,bhkv->bhqv", p, V)
    ctxv = ctxv.transpose(0, 2, 1, 3).reshape(Bsz, Lq, N_HEAD * DV)
    return (ctxv @ Wo + bo).astype(np.float32)


def run_on_hw(in_maps, trace=False):
    from concourse.bass_utils import run_bass_kernel_spmd
    nc = _get_nc()
    return run_bass_kernel_spmd(nc, in_maps, list(range(N_CORES)), trace=trace)


def kernel(x, Wq, Wk, Wv, Wo, bo, mask, _trace=False, _results=None):
    x = np.asarray(x, dtype=np.float32)
    Wq = np.asarray(Wq, dtype=np.float32)
    Wk = np.asarray(Wk, dtype=np.float32)
    Wv = np.asarray(Wv, dtype=np.float32)
    Wo = np.asarray(Wo, dtype=np.float32)
    bo = np.asarray(bo, dtype=np.float32)
    mask_np = np.asarray(mask).reshape(mask.shape[-2], mask.shape[-1])

    causal = bool(np.array_equal(
        mask_np, np.tril(np.ones((L, L), dtype=bool))))
    if not causal or x.shape != (B, L, D):
        return _numpy_fallback(np.asarray(x), Wq, Wk, Wv, Wo, bo,
                               np.asarray(mask))

    res = run_on_hw(make_in_maps(x, Wq, Wk, Wv, Wo), trace=_trace)
    if _results is not None:
        _results.append(res)
    out = np.empty((B, L, D), dtype=np.float32)
    for b in range(B):
        out[b] = (np.asarray(res.results[2 * b]["out"], dtype=np.float32)
                  + np.asarray(res.results[2 * b + 1]["out"], dtype=np.float32)
                  + bo)
    return out



# revision 70
# speedup vs baseline: 1.2103x; 1.2103x over previous
"""Causal multi-head attention Trainium2 kernel (8 NeuronCores).

Problem: B=4, L=2048, D=1024, 16 heads x (dh=64, dv=64), causal mask.
Sharding: data-parallel over batch (4) x tensor-parallel over heads (2 groups
of 8). Core c handles batch c//2, head-group c%2. Each core computes its
partial output projection (ctx_g @ Wo_g); the host sums the two head-group
partials per batch and adds the bias.

v2: software-pipelined. The prologue transposes x (PE) into a resident
xT [d, l] tile, computes V = x@Wv (+ones column for the fused softmax
denominator) and Q^T/K^T for head-pair 0. The main loop runs flash-style
attention per head-pair while dribbling the next head-pair's Q^T/K^T
projection matmuls between attention groups — keeping the PE dense so the
HAM clock gate stays at 2.4 GHz.
S^T tiles = K@Q^T; exp on ACT (scale=1/8 folded in); causal diagonal via one
tril tensor_mul per diag k-tile + width-restricted PV; ones column of V_aug
gives the softmax denominator in PSUM row 64; normalize with DVE
reciprocal_approx_fast + gpsimd partition_broadcast + DVE multiply into the
resident ctx^T tile; output projection at the end.

v3: bf16 datapath (tolerance is 2e-2; halves DMA bytes, LDWEIGHTS stalls,
SBUF traffic; PE matmul rate is 1 cycle/row either way). Causal-restricted
S^T matmul + exp widths on diagonal k-tiles. Wo prefetched right after the
prologue.

v4 (final, ~332us vs 423us baseline): space heater removed (the pipeline
is dense enough that DVFS stays up on real work). Output projection
dribbled into hp3's attention via OutEmitter, leaving only j=3's four
l-tiles for the drain. hp0 runs its q-chunks descending so j=3's 32 S
matmuls hide the cold-ACT exp latency at the prologue boundary. Normalize
copies ctx out of PSUM immediately (pctx bank freed after ~1us instead of
the full recip/broadcast chain, unblocking the next chunk's first PV).
DMA traffic split across queues: bulk weight loads on the ACT hwdge
queue, x slices alternating sync/ACT, ost stores via gpsimd software DGE
- the gpsimd partition_broadcast in the softmax normalize rides sync-queue
DIRECT2D slots and must not sit behind bulk transfers.
"""

import numpy as np
from contextlib import ExitStack

import concourse.bass as bass
import concourse.tile as tile
from concourse import bacc, mybir
from concourse.masks import make_identity

F32 = mybir.dt.float32
F32R = mybir.dt.float32r
BF16 = mybir.dt.bfloat16
AF = mybir.ActivationFunctionType

B, L, D = 4, 2048, 1024
N_HEAD, DH, DV = 16, 64, 64
N_CORES = 8
HPC = N_HEAD // 2          # heads per core (8)
OC = HPC * DH              # per-core projection width (512)
NHP = HPC // 2             # head-pairs per core (4)


class ProjEmitter:
    """Q^T/K^T projection for one head-pair, emitted in per-(proj,chunk)
    units so the matmuls interleave with attention of the previous pair."""

    def __init__(self, nc, hp, pools, xt, wq, wk, nch):
        self.nc = nc
        self.xt = xt
        qkp, wp, self.psP = pools
        self.wq_sb = wp.tile([128, 8, 128], BF16, tag="wq")
        self.wk_sb = wp.tile([128, 8, 128], BF16, tag="wk")
        # ACT hwdge queue: keeps bulk weight loads off the sync queue,
        # whose DIRECT2D slots the normalize partition_broadcasts ride
        nc.scalar.dma_start(
            out=self.wq_sb,
            in_=wq[:, hp * 128:(hp + 1) * 128].rearrange("(t p) o -> p t o", p=128))
        nc.scalar.dma_start(
            out=self.wk_sb,
            in_=wk[:, hp * 128:(hp + 1) * 128].rearrange("(t p) o -> p t o", p=128))
        self.qt = qkp.tile([128, nch * 512], BF16, tag="qt")
        self.kt = qkp.tile([128, nch * 512], BF16, tag="kt")
        self.units = [(w, d, c) for w, d in ((self.wq_sb, self.qt),
                                             (self.wk_sb, self.kt))
                      for c in range(nch)]
        self.i = 0

    def step(self):
        if self.i >= len(self.units):
            return False
        w_sb, dst, c = self.units[self.i]
        self.i += 1
        nc = self.nc
        pp = self.psP.tile([128, 512], F32, tag="pp")
        for d in range(8):
            nc.tensor.matmul(pp, w_sb[:, d, :],
                             self.xt[:, d, c * 512:(c + 1) * 512],
                             start=(d == 0), stop=(d == 7))
        nc.vector.tensor_copy(dst[:, c * 512:(c + 1) * 512], pp)
        return True

    def drain(self):
        while self.step():
            pass


class OutEmitter:
    """Output projection, dribbled into hp3's attention: after head-pair 3
    finishes q-chunk j, the four l-tiles 4j..4j+3 have all heads' ctx ready
    and their out-projection can interleave with the remaining attention."""

    def __init__(self, nc, pools, ct, wo_sb, out):
        self.nc = nc
        self.phco, self.psP = pools
        self.ct, self.wo_sb, self.out = ct, wo_sb, out
        self.queue = []
        self.ost = {}
        self.pa = {}

    def add_lt(self, lt):
        self.queue.extend([("full", lt, 0), ("full", lt, 1)])

    def add_partial(self, lt):
        # v=0..2 contraction only: these l-tiles' head-pair 0-2 ctx is
        # ready before hp3's attention even starts, so 3/4 of the last
        # tiles' projection work dribbles instead of sitting in the drain
        self.queue.extend([("pA", lt, 0), ("pA", lt, 1)])

    def add_finish(self, lt):
        self.queue.extend([("fin", lt, 0), ("fin", lt, 1)])

    def step(self):
        if not self.queue:
            return False
        kind, lt, n = self.queue.pop(0)
        nc = self.nc
        if kind == "pA":
            pp = self.psP.tile([128, 512], F32, tag="pp", name="ppo")
            for v in range(3):
                nc.tensor.matmul(pp, self.ct[:, v, lt * 128:(lt + 1) * 128],
                                 self.wo_sb[:, v, n * 512:(n + 1) * 512],
                                 start=(v == 0), stop=(v == 2))
            pa = self.phco.tile([128, 512], F32, tag="pA", bufs=8,
                                name=f"pA{lt}_{n}")
            nc.vector.tensor_copy(pa, pp)
            self.pa[(lt, n)] = pa
            return True
        if n == 0:
            self.ost[lt] = self.phco.tile([128, D], F32, tag="ost",
                                          name=f"ost{lt}")
        ost = self.ost[lt]
        pp = self.psP.tile([128, 512], F32, tag="pp", name="ppo")
        if kind == "fin":
            nc.tensor.matmul(pp, self.ct[:, 3, lt * 128:(lt + 1) * 128],
                             self.wo_sb[:, 3, n * 512:(n + 1) * 512],
                             start=True, stop=True)
            nc.vector.tensor_add(ost[:, n * 512:(n + 1) * 512], pp,
                                 self.pa.pop((lt, n)))
        else:
            for v in range(4):
                nc.tensor.matmul(pp, self.ct[:, v, lt * 128:(lt + 1) * 128],
                                 self.wo_sb[:, v, n * 512:(n + 1) * 512],
                                 start=(v == 0), stop=(v == 3))
            nc.vector.tensor_copy(ost[:, n * 512:(n + 1) * 512], pp)
        if n == 1:
            # gpsimd software DGE: ost stores would otherwise queue ahead
            # of hp3's normalize broadcasts on the sync queue
            nc.gpsimd.dma_start(out=self.out[lt * 128:(lt + 1) * 128, :],
                                in_=ost)
            del self.ost[lt]
        return True

    def drain(self):
        while self.step():
            pass


def build_nc(l=L):
    assert l % 512 == 0
    nch = l // 512           # q-chunks
    nlt = l // 128           # l-tiles
    nc = bacc.Bacc("TRN2", target_bir_lowering=False, debug=False,
                   num_devices=N_CORES)

    x = nc.dram_tensor("x", [l, D], BF16, kind="ExternalInput").ap()
    wq = nc.dram_tensor("wq", [D, OC], BF16, kind="ExternalInput").ap()
    wk = nc.dram_tensor("wk", [D, OC], BF16, kind="ExternalInput").ap()
    wv = nc.dram_tensor("wv", [D, OC], BF16, kind="ExternalInput").ap()
    wo = nc.dram_tensor("wo", [OC, D], BF16, kind="ExternalInput").ap()
    out = nc.dram_tensor("out", [l, D], F32, kind="ExternalOutput").ap()

    with tile.TileContext(nc) as tc, ExitStack() as ctx:
        top = ctx.enter_context(tc.tile_pool(name="top", bufs=1))
        xtp = ctx.enter_context(tc.tile_pool(name="xtp", bufs=1))
        qkp = ctx.enter_context(tc.tile_pool(name="qkp", bufs=2))
        wp = ctx.enter_context(tc.tile_pool(name="wp", bufs=2))
        phco = ctx.enter_context(tc.tile_pool(name="phco", bufs=3))

        # V: [128(l), ltile, head, 65] - col 64 is ones (softmax denominator)
        vt = top.tile([128, nlt, HPC, DH + 1], BF16)
        ct = top.tile([128, NHP, l], BF16)        # normalized ctx^T
        tril = top.tile([128, 128], BF16)
        ones = top.tile([128, 1], F32)
        warm = top.tile([128, 1], BF16)
        onesw = top.tile([1, DV], BF16)           # rank-1 bcast weights
        xt = xtp.tile([128, 8, l], BF16)          # x^T, d-major

        nc.vector.memset(ones, 1.0)
        nc.vector.memset(onesw, 1.0)
        # warm-up exp: loads the ACT function table during the DMA-bound
        # startup instead of stalling the first attention group
        nc.scalar.activation(warm, ones, AF.Exp, scale=0.125)
        nc.vector.tensor_copy(
            vt[:, :, :, DV:DV + 1].rearrange("p t h c -> p (t h) c"),
            ones.broadcast_to((128, nlt * HPC, 1)))
        # causal keep-mask for S^T diag blocks: tril[k, q] = 1.0 iff q >= k
        nc.gpsimd.memset(tril, 0.0)
        nc.gpsimd.affine_select(
            out=tril, in_=tril, compare_op=mybir.AluOpType.is_gt,
            fill=1.0, base=0, pattern=[[-1, 128]], channel_multiplier=1)

        # ---------------- Prologue: transpose + V + QK(hp=0) --------------
        with tc.tile_pool(name="pro", bufs=6) as pro, \
             tc.tile_pool(name="wvp", bufs=1) as wvp, \
             tc.tile_pool(name="psPro", bufs=2, space="PSUM") as psPro, \
             tc.tile_pool(name="psT", bufs=3, space="PSUM") as psT:
            ident = wvp.tile([128, 128], BF16)
            make_identity(nc, ident)
            # chunk-0 x slices first so the first transposes start ASAP;
            # only then the Wv load (needed one chunk later)
            # x slices alternate across both hwdge queues
            xst0 = []
            for s in range(4):
                xst = pro.tile([128, D], BF16, tag="xst", name=f"xst0{s}")
                eng = nc.sync if s % 2 == 0 else nc.scalar
                eng.dma_start(out=xst, in_=x[s * 128:(s + 1) * 128, :])
                xst0.append(xst)
            wv_sb = wvp.tile([128, 8, OC], BF16)
            nc.scalar.dma_start(out=wv_sb,
                                in_=wv.rearrange("(t p) o -> p t o", p=128))
            for c in range(nch):
                for s in range(4):
                    if c == 0:
                        xst = xst0[s]
                    else:
                        xst = pro.tile([128, D], BF16, tag="xst")
                        eng = nc.sync if s % 2 == 0 else nc.scalar
                        eng.dma_start(
                            out=xst,
                            in_=x[c * 512 + s * 128: c * 512 + (s + 1) * 128, :])
                    # all 8 d-blocks transpose into one 1-bank PSUM tile,
                    # drained by a single wide DVE copy (8x fewer copies)
                    pt8 = psT.tile([128, 8, 128], BF16, tag="pt")
                    for d in range(8):
                        nc.tensor.transpose(
                            pt8[:, d, :], xst[:, d * 128:(d + 1) * 128],
                            ident)
                    nc.vector.tensor_copy(
                        xt[:, :, c * 512 + s * 128: c * 512 + (s + 1) * 128],
                        pt8)
                # V for this l-chunk
                for m in range(4):
                    pp = psPro.tile([128, 512], F32, tag="pp")
                    for d in range(8):
                        nc.tensor.matmul(
                            pp, xt[:, d, (c * 4 + m) * 128:(c * 4 + m + 1) * 128],
                            wv_sb[:, d, :], start=(d == 0), stop=(d == 7))
                    nc.vector.tensor_copy(
                        vt[:, c * 4 + m, :, 0:DV],
                        pp.rearrange("p (h v) -> p h v", h=HPC))
                if c == 1:
                    # issue hp0's Wq/Wk DMAs mid-prologue so the QK drain
                    # at the end doesn't stall on them
                    em = ProjEmitter(nc, 0, (qkp, wp, psPro), xt, wq, wk, nch)
            em.drain()

        # Prefetch Wo now: the DMA queue is idle during attention, and the
        # output projection otherwise stalls ~14us on this load at the end.
        phc = ctx.enter_context(tc.tile_pool(name="phc", bufs=1))
        wo_sb = phc.tile([128, 4, D], BF16)
        nc.scalar.dma_start(out=wo_sb,
                            in_=wo.rearrange("(t p) o -> p t o", p=128))

        # ---------------- Main: attention + next-pair projections ---------
        with tc.tile_pool(name="phb", bufs=2) as phb, \
             tc.tile_pool(name="psS", bufs=2, space="PSUM") as psS, \
             tc.tile_pool(name="psPd", bufs=2, space="PSUM") as psPd, \
             tc.tile_pool(name="psC", bufs=2, space="PSUM") as psC:
            n_groups_hp = 2 * nch * (nch + 1)
            for hp in range(NHP):
                qt, kt = em.qt, em.kt
                if hp + 1 < NHP:
                    em = ProjEmitter(nc, hp + 1, (qkp, wp, psPd), xt, wq, wk,
                                     nch)
                    cadence = max(1, (n_groups_hp // 2) // (2 * nch))
                else:
                    em = OutEmitter(nc, (phco, psPd), ct, wo_sb, out)
                    cadence = 1
                gcount = 0

                def pv_step(g, j, pctx, pexp, po, H):
                    # masks + PV for group g (one group after its exp)
                    for r2 in range(2):
                        kt_i = 2 * g + r2
                        r = kt_i - 4 * j
                        c0 = 0
                        if r >= 0:      # diagonal k-tile
                            c0 = r * 128
                            nc.vector.tensor_mul(
                                pexp[:, r2, c0:c0 + 128],
                                pexp[:, r2, c0:c0 + 128], tril)
                        nc.tensor.matmul(
                            pctx[:, c0:512],
                            vt[:, kt_i, H, :],
                            pexp[:, r2, c0:512],
                            start=(kt_i == 0), stop=(kt_i == 4 * j + 3))

                # both heads interleaved at group level: two independent
                # dependency chains keep the PE busy through each other's
                # exp waits. hp0 runs j descending: at the prologue
                # boundary the ACT pipeline is cold, and j=3 offers 32 S
                # matmuls to hide the first exp latencies (j=0 only 8).
                jorder = range(nch - 1, -1, -1) if hp == 0 else range(nch)
                for j in jorder:
                    if hp == NHP - 1:
                        # final j: spread the remaining dribble units so
                        # some out-proj work still sits in the PE queue
                        # while the last normalize chain runs
                        cadence = 2 if j == nch - 1 else 1
                    n_g = 2 * (j + 1)
                    pctxs = {}
                    prevs = {0: None, 1: None}
                    for h in range(2):
                        pctxs[h] = psC.tile([DV + 1, 512], F32,
                                            tag="pctx", name=f"pctx{h}")
                    for g in range(n_g + 1):
                        for h in range(2):
                            po = 64 * h
                            H = 2 * hp + h
                            pexp = None
                            if g < n_g:
                                psc = psS.tile([128, 2, 512], F32,
                                               tag="psc", name=f"psc{h}")
                                for r2 in range(2):
                                    kt_i = 2 * g + r2
                                    # causal: diag k-tile kt_i only feeds
                                    # q columns >= (kt_i-4j)*128
                                    c0 = max(0, kt_i - 4 * j) * 128
                                    nc.tensor.matmul(
                                        psc[:, r2, c0:512],
                                        kt[po:po + DH,
                                           kt_i * 128:(kt_i + 1) * 128],
                                        qt[po:po + DH,
                                           j * 512 + c0:(j + 1) * 512],
                                        start=True, stop=True)
                                pexp = phb.tile([128, 2, 512], BF16,
                                                tag="pexp", bufs=10,
                                                name=f"pexp{h}")
                                cg = max(0, 2 * g - 4 * j) * 128
                                nc.scalar.activation(pexp[:, :, cg:512],
                                                     psc[:, :, cg:512],
                                                     AF.Exp, scale=0.125)
                            if prevs[h] is not None:
                                pv_step(prevs[h][0], j, pctxs[h],
                                        prevs[h][1], po, H)
                                gcount += 1
                                if em and cadence and gcount % cadence == 0:
                                    em.step()
                            prevs[h] = (g, pexp) if g < n_g else None
                    for h in range(2):
                        po = 64 * h
                        # free the pctx PSUM bank after two quick DVE copies
                        # (the next j's first PV waits on it); the
                        # recip/broadcast/scale dangle off the critical path.
                        # ctu is a base-0 staging tile so the scale-mul's two
                        # SBUF inputs share a base partition.
                        rs = phb.tile([1, 512], F32, tag="rs", name="rs")
                        nc.vector.tensor_copy(rs, pctxs[h][DV:DV + 1, :])
                        ctu = phb.tile([64, 512], BF16, tag="ctu",
                                       name=f"ctu{h}")
                        nc.vector.tensor_copy(ctu, pctxs[h][0:DV, :])
                        inv = phb.tile([1, 512], F32, tag="inv", name="inv")
                        nc.vector.reciprocal_approx_fast(out=inv, in_=rs)
                        if hp == NHP - 1 and j == nch - 1:
                            # last chunk: the out-proj drain waits on this
                            # chain, so broadcast via a rank-1 PE matmul
                            # (~0.2us, PE is idling here) instead of the
                            # ~1us gpsimd DIRECT2D path
                            invb = phb.tile([1, 512], BF16, tag="invb",
                                            name="invb")
                            nc.vector.tensor_copy(invb, inv)
                            bcp = psPd.tile([128, 512], F32, tag="pp",
                                            name="bcp")
                            nc.tensor.matmul(bcp[0:DV, :], onesw, invb,
                                             start=True, stop=True)
                            nc.vector.tensor_mul(
                                ct[po:po + DV, hp, j * 512:(j + 1) * 512],
                                ctu, bcp[0:DV, :])
                        else:
                            bc = phb.tile([64, 512], F32, tag="bc",
                                          name="bc")
                            nc.gpsimd.partition_broadcast(out_ap=bc,
                                                          in_ap=inv)
                            nc.vector.tensor_mul(
                                ct[po:po + DV, hp, j * 512:(j + 1) * 512],
                                ctu, bc)
                    if hp == NHP - 1:
                        # all heads' ctx for l-tiles 4j..4j+3 now ready;
                        # queue their output projection for dribbling
                        for lt in range(4 * j, 4 * j + 4):
                            em.add_lt(lt)
                em.drain()

    nc.compile()
    return nc


def _bf16(a):
    import ml_dtypes
    return np.ascontiguousarray(a).astype(ml_dtypes.bfloat16)


def make_in_maps(x, Wq, Wk, Wv, Wo):
    in_maps = []
    for c in range(N_CORES):
        b, g = c // 2, c % 2
        in_maps.append({
            "x": _bf16(x[b]),
            "wq": _bf16(Wq[:, g * OC:(g + 1) * OC]),
            "wk": _bf16(Wk[:, g * OC:(g + 1) * OC]),
            "wv": _bf16(Wv[:, g * OC:(g + 1) * OC]),
            "wo": _bf16(Wo[g * OC:(g + 1) * OC, :]),
        })
    return in_maps


_NC_CACHE = {}


def _get_nc():
    if "nc" not in _NC_CACHE:
        _NC_CACHE["nc"] = build_nc()
    return _NC_CACHE["nc"]


def _numpy_fallback(x, Wq, Wk, Wv, Wo, bo, mask):
    Bsz, Lq, _ = x.shape
    Q = (x @ Wq).reshape(Bsz, Lq, N_HEAD, DH).transpose(0, 2, 1, 3)
    K = (x @ Wk).reshape(Bsz, Lq, N_HEAD, DH).transpose(0, 2, 1, 3)
    V = (x @ Wv).reshape(Bsz, Lq, N_HEAD, DV).transpose(0, 2, 1, 3)
    s = np.einsum("bhqd,bhkd->bhqk", Q, K) / np.sqrt(np.float32(DH))
    s = np.where(mask, s, -np.inf)
    s = s - s.max(axis=-1, keepdims=True)
    p = np.exp(s)
    p /= p.sum(axis=-1, keepdims=True)
    ctxv = np.einsum("bhqk,bhkv->bhqv", p, V)
    ctxv = ctxv.transpose(0, 2, 1, 3).reshape(Bsz, Lq, N_HEAD * DV)
    return (ctxv @ Wo + bo).astype(np.float32)


def run_on_hw(in_maps, trace=False):
    from concourse.bass_utils import run_bass_kernel_spmd
    nc = _get_nc()
    return run_bass_kernel_spmd(nc, in_maps, list(range(N_CORES)), trace=trace)


def kernel(x, Wq, Wk, Wv, Wo, bo, mask, _trace=False, _results=None):
    x = np.asarray(x, dtype=np.float32)
    Wq = np.asarray(Wq, dtype=np.float32)
    Wk = np.asarray(Wk, dtype=np.float32)
    Wv = np.asarray(Wv, dtype=np.float32)
    Wo = np.asarray(Wo, dtype=np.float32)
    bo = np.asarray(bo, dtype=np.float32)
    mask_np = np.asarray(mask).reshape(mask.shape[-2], mask.shape[-1])

    causal = bool(np.array_equal(
        mask_np, np.tril(np.ones((L, L), dtype=bool))))
    if not causal or x.shape != (B, L, D):
        return _numpy_fallback(np.asarray(x), Wq, Wk, Wv, Wo, bo,
                               np.asarray(mask))

    res = run_on_hw(make_in_maps(x, Wq, Wk, Wv, Wo), trace=_trace)
    if _results is not None:
        _results.append(res)
    out = np.empty((B, L, D), dtype=np.float32)
    for b in range(B):
        out[b] = (np.asarray(res.results[2 * b]["out"], dtype=np.float32)
                  + np.asarray(res.results[2 * b + 1]["out"], dtype=np.float32)
                  + bo)
    return out



# revision 71
# speedup vs baseline: 1.2118x; 1.0012x over previous
"""Causal multi-head attention Trainium2 kernel (8 NeuronCores).

Problem: B=4, L=2048, D=1024, 16 heads x (dh=64, dv=64), causal mask.
Sharding: data-parallel over batch (4) x tensor-parallel over heads (2 groups
of 8). Core c handles batch c//2, head-group c%2. Each core computes its
partial output projection (ctx_g @ Wo_g); the host sums the two head-group
partials per batch and adds the bias.

v2: software-pipelined. The prologue transposes x (PE) into a resident
xT [d, l] tile, computes V = x@Wv (+ones column for the fused softmax
denominator) and Q^T/K^T for head-pair 0. The main loop runs flash-style
attention per head-pair while dribbling the next head-pair's Q^T/K^T
projection matmuls between attention groups — keeping the PE dense so the
HAM clock gate stays at 2.4 GHz.
S^T tiles = K@Q^T; exp on ACT (scale=1/8 folded in); causal diagonal via one
tril tensor_mul per diag k-tile + width-restricted PV; ones column of V_aug
gives the softmax denominator in PSUM row 64; normalize with DVE
reciprocal_approx_fast + gpsimd partition_broadcast + DVE multiply into the
resident ctx^T tile; output projection at the end.

v3: bf16 datapath (tolerance is 2e-2; halves DMA bytes, LDWEIGHTS stalls,
SBUF traffic; PE matmul rate is 1 cycle/row either way). Causal-restricted
S^T matmul + exp widths on diagonal k-tiles. Wo prefetched right after the
prologue.

v4 (final, ~332us vs 423us baseline): space heater removed (the pipeline
is dense enough that DVFS stays up on real work). Output projection
dribbled into hp3's attention via OutEmitter, leaving only j=3's four
l-tiles for the drain. hp0 runs its q-chunks descending so j=3's 32 S
matmuls hide the cold-ACT exp latency at the prologue boundary. Normalize
copies ctx out of PSUM immediately (pctx bank freed after ~1us instead of
the full recip/broadcast chain, unblocking the next chunk's first PV).
DMA traffic split across queues: bulk weight loads on the ACT hwdge
queue, x slices alternating sync/ACT, ost stores via gpsimd software DGE
- the gpsimd partition_broadcast in the softmax normalize rides sync-queue
DIRECT2D slots and must not sit behind bulk transfers.
"""

import numpy as np
from contextlib import ExitStack

import concourse.bass as bass
import concourse.tile as tile
from concourse import bacc, mybir
from concourse.masks import make_identity

F32 = mybir.dt.float32
F32R = mybir.dt.float32r
BF16 = mybir.dt.bfloat16
AF = mybir.ActivationFunctionType

B, L, D = 4, 2048, 1024
N_HEAD, DH, DV = 16, 64, 64
N_CORES = 8
HPC = N_HEAD // 2          # heads per core (8)
OC = HPC * DH              # per-core projection width (512)
NHP = HPC // 2             # head-pairs per core (4)


class ProjEmitter:
    """Q^T/K^T projection for one head-pair, emitted in per-(proj,chunk)
    units so the matmuls interleave with attention of the previous pair."""

    def __init__(self, nc, hp, pools, xt, wq, wk, nch):
        self.nc = nc
        self.xt = xt
        qkp, wp, self.psP = pools
        self.wq_sb = wp.tile([128, 8, 128], BF16, tag="wq")
        self.wk_sb = wp.tile([128, 8, 128], BF16, tag="wk")
        # ACT hwdge queue: keeps bulk weight loads off the sync queue,
        # whose DIRECT2D slots the normalize partition_broadcasts ride
        nc.scalar.dma_start(
            out=self.wq_sb,
            in_=wq[:, hp * 128:(hp + 1) * 128].rearrange("(t p) o -> p t o", p=128))
        nc.scalar.dma_start(
            out=self.wk_sb,
            in_=wk[:, hp * 128:(hp + 1) * 128].rearrange("(t p) o -> p t o", p=128))
        self.qt = qkp.tile([128, nch * 512], BF16, tag="qt")
        self.kt = qkp.tile([128, nch * 512], BF16, tag="kt")
        self.units = [(w, d, c) for w, d in ((self.wq_sb, self.qt),
                                             (self.wk_sb, self.kt))
                      for c in range(nch)]
        self.i = 0

    def step(self):
        if self.i >= len(self.units):
            return False
        w_sb, dst, c = self.units[self.i]
        self.i += 1
        nc = self.nc
        pp = self.psP.tile([128, 512], F32, tag="pp")
        for d in range(8):
            nc.tensor.matmul(pp, w_sb[:, d, :],
                             self.xt[:, d, c * 512:(c + 1) * 512],
                             start=(d == 0), stop=(d == 7))
        nc.vector.tensor_copy(dst[:, c * 512:(c + 1) * 512], pp)
        return True

    def drain(self):
        while self.step():
            pass


class OutEmitter:
    """Output projection, dribbled into hp3's attention: after head-pair 3
    finishes q-chunk j, the four l-tiles 4j..4j+3 have all heads' ctx ready
    and their out-projection can interleave with the remaining attention."""

    def __init__(self, nc, pools, ct, wo_sb, out):
        self.nc = nc
        self.phco, self.psP = pools
        self.ct, self.wo_sb, self.out = ct, wo_sb, out
        self.queue = []
        self.ost = {}
        self.pa = {}

    def add_lt(self, lt):
        self.queue.extend([("full", lt, 0), ("full", lt, 1)])

    def add_partial(self, lt):
        # v=0..2 contraction only: these l-tiles' head-pair 0-2 ctx is
        # ready before hp3's attention even starts, so 3/4 of the last
        # tiles' projection work dribbles instead of sitting in the drain
        self.queue.extend([("pA", lt, 0), ("pA", lt, 1)])

    def add_finish(self, lt):
        self.queue.extend([("fin", lt, 0), ("fin", lt, 1)])

    def step(self):
        if not self.queue:
            return False
        kind, lt, n = self.queue.pop(0)
        nc = self.nc
        if kind == "pA":
            pp = self.psP.tile([128, 512], F32, tag="pp", name="ppo")
            for v in range(3):
                nc.tensor.matmul(pp, self.ct[:, v, lt * 128:(lt + 1) * 128],
                                 self.wo_sb[:, v, n * 512:(n + 1) * 512],
                                 start=(v == 0), stop=(v == 2))
            pa = self.phco.tile([128, 512], F32, tag="pA", bufs=8,
                                name=f"pA{lt}_{n}")
            nc.vector.tensor_copy(pa, pp)
            self.pa[(lt, n)] = pa
            return True
        if n == 0:
            self.ost[lt] = self.phco.tile([128, D], F32, tag="ost",
                                          name=f"ost{lt}")
        ost = self.ost[lt]
        pp = self.psP.tile([128, 512], F32, tag="pp", name="ppo")
        if kind == "fin":
            nc.tensor.matmul(pp, self.ct[:, 3, lt * 128:(lt + 1) * 128],
                             self.wo_sb[:, 3, n * 512:(n + 1) * 512],
                             start=True, stop=True)
            nc.vector.tensor_add(ost[:, n * 512:(n + 1) * 512], pp,
                                 self.pa.pop((lt, n)))
        else:
            for v in range(4):
                nc.tensor.matmul(pp, self.ct[:, v, lt * 128:(lt + 1) * 128],
                                 self.wo_sb[:, v, n * 512:(n + 1) * 512],
                                 start=(v == 0), stop=(v == 3))
            nc.vector.tensor_copy(ost[:, n * 512:(n + 1) * 512], pp)
        if n == 1:
            # gpsimd software DGE: ost stores would otherwise queue ahead
            # of hp3's normalize broadcasts on the sync queue
            nc.gpsimd.dma_start(out=self.out[lt * 128:(lt + 1) * 128, :],
                                in_=ost)
            del self.ost[lt]
        return True

    def drain(self):
        while self.step():
            pass


def build_nc(l=L):
    assert l % 512 == 0
    nch = l // 512           # q-chunks
    nlt = l // 128           # l-tiles
    nc = bacc.Bacc("TRN2", target_bir_lowering=False, debug=False,
                   num_devices=N_CORES)

    x = nc.dram_tensor("x", [l, D], BF16, kind="ExternalInput").ap()
    wq = nc.dram_tensor("wq", [D, OC], BF16, kind="ExternalInput").ap()
    wk = nc.dram_tensor("wk", [D, OC], BF16, kind="ExternalInput").ap()
    wv = nc.dram_tensor("wv", [D, OC], BF16, kind="ExternalInput").ap()
    wo = nc.dram_tensor("wo", [OC, D], BF16, kind="ExternalInput").ap()
    out = nc.dram_tensor("out", [l, D], F32, kind="ExternalOutput").ap()

    with tile.TileContext(nc) as tc, ExitStack() as ctx:
        top = ctx.enter_context(tc.tile_pool(name="top", bufs=1))
        xtp = ctx.enter_context(tc.tile_pool(name="xtp", bufs=1))
        qkp = ctx.enter_context(tc.tile_pool(name="qkp", bufs=2))
        wp = ctx.enter_context(tc.tile_pool(name="wp", bufs=2))
        phco = ctx.enter_context(tc.tile_pool(name="phco", bufs=3))

        # V: [128(l), ltile, head, 65] - col 64 is ones (softmax denominator)
        vt = top.tile([128, nlt, HPC, DH + 1], BF16)
        ct = top.tile([128, NHP, l], BF16)        # normalized ctx^T
        tril = top.tile([128, 128], BF16)
        ones = top.tile([128, 1], F32)
        warm = top.tile([128, 1], BF16)
        onesw = top.tile([1, DV], BF16)           # rank-1 bcast weights
        xt = xtp.tile([128, 8, l], BF16)          # x^T, d-major

        nc.vector.memset(ones, 1.0)
        nc.vector.memset(onesw, 1.0)
        # warm-up exp: loads the ACT function table during the DMA-bound
        # startup instead of stalling the first attention group
        nc.scalar.activation(warm, ones, AF.Exp, scale=0.125)
        nc.vector.tensor_copy(
            vt[:, :, :, DV:DV + 1].rearrange("p t h c -> p (t h) c"),
            ones.broadcast_to((128, nlt * HPC, 1)))
        # causal keep-mask for S^T diag blocks: tril[k, q] = 1.0 iff q >= k
        nc.gpsimd.memset(tril, 0.0)
        nc.gpsimd.affine_select(
            out=tril, in_=tril, compare_op=mybir.AluOpType.is_gt,
            fill=1.0, base=0, pattern=[[-1, 128]], channel_multiplier=1)

        # ---------------- Prologue: transpose + V + QK(hp=0) --------------
        with tc.tile_pool(name="pro", bufs=6) as pro, \
             tc.tile_pool(name="wvp", bufs=1) as wvp, \
             tc.tile_pool(name="psPro", bufs=2, space="PSUM") as psPro, \
             tc.tile_pool(name="psT", bufs=3, space="PSUM") as psT:
            ident = wvp.tile([128, 128], BF16)
            make_identity(nc, ident)
            # chunk-0 x slices first so the first transposes start ASAP;
            # only then the Wv load (needed one chunk later)
            # Wv leads the scalar queue (V(c0) waits on it) while chunk 0's
            # x slices stream on sync; later chunks alternate both queues
            wv_sb = wvp.tile([128, 8, OC], BF16)
            nc.scalar.dma_start(out=wv_sb,
                                in_=wv.rearrange("(t p) o -> p t o", p=128))
            xst0 = []
            for s in range(4):
                xst = pro.tile([128, D], BF16, tag="xst", name=f"xst0{s}")
                nc.sync.dma_start(out=xst, in_=x[s * 128:(s + 1) * 128, :])
                xst0.append(xst)
            for c in range(nch):
                for s in range(4):
                    if c == 0:
                        xst = xst0[s]
                    else:
                        xst = pro.tile([128, D], BF16, tag="xst")
                        eng = nc.sync if s % 2 == 0 else nc.scalar
                        eng.dma_start(
                            out=xst,
                            in_=x[c * 512 + s * 128: c * 512 + (s + 1) * 128, :])
                    # all 8 d-blocks transpose into one 1-bank PSUM tile,
                    # drained by a single wide DVE copy (8x fewer copies)
                    pt8 = psT.tile([128, 8, 128], BF16, tag="pt")
                    for d in range(8):
                        nc.tensor.transpose(
                            pt8[:, d, :], xst[:, d * 128:(d + 1) * 128],
                            ident)
                    nc.vector.tensor_copy(
                        xt[:, :, c * 512 + s * 128: c * 512 + (s + 1) * 128],
                        pt8)
                # V for this l-chunk
                for m in range(4):
                    pp = psPro.tile([128, 512], F32, tag="pp")
                    for d in range(8):
                        nc.tensor.matmul(
                            pp, xt[:, d, (c * 4 + m) * 128:(c * 4 + m + 1) * 128],
                            wv_sb[:, d, :], start=(d == 0), stop=(d == 7))
                    nc.vector.tensor_copy(
                        vt[:, c * 4 + m, :, 0:DV],
                        pp.rearrange("p (h v) -> p h v", h=HPC))
                if c == 1:
                    # issue hp0's Wq/Wk DMAs mid-prologue so the QK drain
                    # at the end doesn't stall on them
                    em = ProjEmitter(nc, 0, (qkp, wp, psPro), xt, wq, wk, nch)
            em.drain()

        # Prefetch Wo now: the DMA queue is idle during attention, and the
        # output projection otherwise stalls ~14us on this load at the end.
        phc = ctx.enter_context(tc.tile_pool(name="phc", bufs=1))
        wo_sb = phc.tile([128, 4, D], BF16)
        nc.scalar.dma_start(out=wo_sb,
                            in_=wo.rearrange("(t p) o -> p t o", p=128))

        # ---------------- Main: attention + next-pair projections ---------
        with tc.tile_pool(name="phb", bufs=2) as phb, \
             tc.tile_pool(name="psS", bufs=2, space="PSUM") as psS, \
             tc.tile_pool(name="psPd", bufs=2, space="PSUM") as psPd, \
             tc.tile_pool(name="psC", bufs=2, space="PSUM") as psC:
            n_groups_hp = 2 * nch * (nch + 1)
            for hp in range(NHP):
                qt, kt = em.qt, em.kt
                if hp + 1 < NHP:
                    em = ProjEmitter(nc, hp + 1, (qkp, wp, psPd), xt, wq, wk,
                                     nch)
                    cadence = max(1, (n_groups_hp // 2) // (2 * nch))
                else:
                    em = OutEmitter(nc, (phco, psPd), ct, wo_sb, out)
                    cadence = 1
                gcount = 0

                def pv_step(g, j, pctx, pexp, po, H):
                    # masks + PV for group g (one group after its exp)
                    for r2 in range(2):
                        kt_i = 2 * g + r2
                        r = kt_i - 4 * j
                        c0 = 0
                        if r >= 0:      # diagonal k-tile
                            c0 = r * 128
                            nc.vector.tensor_mul(
                                pexp[:, r2, c0:c0 + 128],
                                pexp[:, r2, c0:c0 + 128], tril)
                        nc.tensor.matmul(
                            pctx[:, c0:512],
                            vt[:, kt_i, H, :],
                            pexp[:, r2, c0:512],
                            start=(kt_i == 0), stop=(kt_i == 4 * j + 3))

                # both heads interleaved at group level: two independent
                # dependency chains keep the PE busy through each other's
                # exp waits. hp0 runs j descending: at the prologue
                # boundary the ACT pipeline is cold, and j=3 offers 32 S
                # matmuls to hide the first exp latencies (j=0 only 8).
                jorder = range(nch - 1, -1, -1) if hp == 0 else range(nch)
                for j in jorder:
                    if hp == NHP - 1:
                        # final j: spread the remaining dribble units so
                        # some out-proj work still sits in the PE queue
                        # while the last normalize chain runs
                        cadence = 2 if j == nch - 1 else 1
                    n_g = 2 * (j + 1)
                    pctxs = {}
                    prevs = {0: None, 1: None}
                    for h in range(2):
                        pctxs[h] = psC.tile([DV + 1, 512], F32,
                                            tag="pctx", name=f"pctx{h}")
                    for g in range(n_g + 1):
                        for h in range(2):
                            po = 64 * h
                            H = 2 * hp + h
                            pexp = None
                            if g < n_g:
                                psc = psS.tile([128, 2, 512], F32,
                                               tag="psc", name=f"psc{h}")
                                for r2 in range(2):
                                    kt_i = 2 * g + r2
                                    # causal: diag k-tile kt_i only feeds
                                    # q columns >= (kt_i-4j)*128
                                    c0 = max(0, kt_i - 4 * j) * 128
                                    nc.tensor.matmul(
                                        psc[:, r2, c0:512],
                                        kt[po:po + DH,
                                           kt_i * 128:(kt_i + 1) * 128],
                                        qt[po:po + DH,
                                           j * 512 + c0:(j + 1) * 512],
                                        start=True, stop=True)
                                pexp = phb.tile([128, 2, 512], BF16,
                                                tag="pexp", bufs=10,
                                                name=f"pexp{h}")
                                cg = max(0, 2 * g - 4 * j) * 128
                                nc.scalar.activation(pexp[:, :, cg:512],
                                                     psc[:, :, cg:512],
                                                     AF.Exp, scale=0.125)
                            if prevs[h] is not None:
                                pv_step(prevs[h][0], j, pctxs[h],
                                        prevs[h][1], po, H)
                                gcount += 1
                                if em and cadence and gcount % cadence == 0:
                                    em.step()
                            prevs[h] = (g, pexp) if g < n_g else None
                    for h in range(2):
                        po = 64 * h
                        # free the pctx PSUM bank after two quick DVE copies
                        # (the next j's first PV waits on it); the
                        # recip/broadcast/scale dangle off the critical path.
                        # ctu is a base-0 staging tile so the scale-mul's two
                        # SBUF inputs share a base partition.
                        rs = phb.tile([1, 512], F32, tag="rs", name="rs")
                        nc.vector.tensor_copy(rs, pctxs[h][DV:DV + 1, :])
                        ctu = phb.tile([64, 512], BF16, tag="ctu",
                                       name=f"ctu{h}")
                        nc.vector.tensor_copy(ctu, pctxs[h][0:DV, :])
                        inv = phb.tile([1, 512], F32, tag="inv", name="inv")
                        nc.vector.reciprocal_approx_fast(out=inv, in_=rs)
                        if hp == NHP - 1 and j == nch - 1:
                            # last chunk: the out-proj drain waits on this
                            # chain, so broadcast via a rank-1 PE matmul
                            # (~0.2us, PE is idling here) instead of the
                            # ~1us gpsimd DIRECT2D path
                            invb = phb.tile([1, 512], BF16, tag="invb",
                                            name="invb")
                            nc.vector.tensor_copy(invb, inv)
                            bcp = psPd.tile([128, 512], F32, tag="pp",
                                            name="bcp")
                            nc.tensor.matmul(bcp[0:DV, :], onesw, invb,
                                             start=True, stop=True)
                            nc.vector.tensor_mul(
                                ct[po:po + DV, hp, j * 512:(j + 1) * 512],
                                ctu, bcp[0:DV, :])
                        else:
                            bc = phb.tile([64, 512], F32, tag="bc",
                                          name="bc")
                            nc.gpsimd.partition_broadcast(out_ap=bc,
                                                          in_ap=inv)
                            nc.vector.tensor_mul(
                                ct[po:po + DV, hp, j * 512:(j + 1) * 512],
                                ctu, bc)
                    if hp == NHP - 1:
                        # all heads' ctx for l-tiles 4j..4j+3 now ready;
                        # queue their output projection for dribbling
                        for lt in range(4 * j, 4 * j + 4):
                            em.add_lt(lt)
                em.drain()

    nc.compile()
    return nc


def _bf16(a):
    import ml_dtypes
    return np.ascontiguousarray(a).astype(ml_dtypes.bfloat16)


def make_in_maps(x, Wq, Wk, Wv, Wo):
    in_maps = []
    for c in range(N_CORES):
        b, g = c // 2, c % 2
        in_maps.append({
            "x": _bf16(x[b]),
            "wq": _bf16(Wq[:, g * OC:(g + 1) * OC]),
            "wk": _bf16(Wk[:, g * OC:(g + 1) * OC]),
            "wv": _bf16(Wv[:, g * OC:(g + 1) * OC]),
            "wo": _bf16(Wo[g * OC:(g + 1) * OC, :]),
        })
    return in_maps


_NC_CACHE = {}


def _get_nc():
    if "nc" not in _NC_CACHE:
        _NC_CACHE["nc"] = build_nc()
    return _NC_CACHE["nc"]


def _numpy_fallback(x, Wq, Wk, Wv, Wo, bo, mask):
    Bsz, Lq, _ = x.shape
    Q = (x @ Wq).reshape(Bsz, Lq, N_HEAD, DH).transpose(0, 2, 1, 3)
    K = (x @ Wk).reshape(Bsz, Lq, N_HEAD, DH).transpose(0, 2, 1, 3)
    V = (x @ Wv).reshape(Bsz, Lq, N_HEAD, DV).transpose(0, 2, 1, 3)
    s = np.einsum("bhqd,bhkd->bhqk", Q, K) / np.sqrt(np.float32(DH))
    s = np.where(mask, s, -np.inf)
    s = s - s.max(axis=-1, keepdims=True)
    p = np.exp(s)
    p /= p.sum(axis=-1, keepdims=True)
    ctxv = np.einsum("bhqk,bhkv->bhqv", p, V)
    ctxv = ctxv.transpose(0, 2, 1, 3).reshape(Bsz, Lq, N_HEAD * DV)
    return (ctxv @ Wo + bo).astype(np.float32)


def run_on_hw(in_maps, trace=False):
    from concourse.bass_utils import run_bass_kernel_spmd
    nc = _get_nc()
    return run_bass_kernel_spmd(nc, in_maps, list(range(N_CORES)), trace=trace)


def kernel(x, Wq, Wk, Wv, Wo, bo, mask, _trace=False, _results=None):
    x = np.asarray(x, dtype=np.float32)
    Wq = np.asarray(Wq, dtype=np.float32)
    Wk = np.asarray(Wk, dtype=np.float32)
    Wv = np.asarray(Wv, dtype=np.float32)
    Wo = np.asarray(Wo, dtype=np.float32)
    bo = np.asarray(bo, dtype=np.float32)
    mask_np = np.asarray(mask).reshape(mask.shape[-2], mask.shape[-1])

    causal = bool(np.array_equal(
        mask_np, np.tril(np.ones((L, L), dtype=bool))))
    if not causal or x.shape != (B, L, D):
        return _numpy_fallback(np.asarray(x), Wq, Wk, Wv, Wo, bo,
                               np.asarray(mask))

    res = run_on_hw(make_in_maps(x, Wq, Wk, Wv, Wo), trace=_trace)
    if _results is not None:
        _results.append(res)
    out = np.empty((B, L, D), dtype=np.float32)
    for b in range(B):
        out[b] = (np.asarray(res.results[2 * b]["out"], dtype=np.float32)
                  + np.asarray(res.results[2 * b + 1]["out"], dtype=np.float32)
                  + bo)
    return out



# revision 72
# speedup vs baseline: 1.2170x; 1.0043x over previous
"""Causal multi-head attention Trainium2 kernel (8 NeuronCores).

Problem: B=4, L=2048, D=1024, 16 heads x (dh=64, dv=64), causal mask.
Sharding: data-parallel over batch (4) x tensor-parallel over heads (2 groups
of 8). Core c handles batch c//2, head-group c%2. Each core computes its
partial output projection (ctx_g @ Wo_g); the host sums the two head-group
partials per batch and adds the bias.

v2: software-pipelined. The prologue transposes x (PE) into a resident
xT [d, l] tile, computes V = x@Wv (+ones column for the fused softmax
denominator) and Q^T/K^T for head-pair 0. The main loop runs flash-style
attention per head-pair while dribbling the next head-pair's Q^T/K^T
projection matmuls between attention groups — keeping the PE dense so the
HAM clock gate stays at 2.4 GHz.
S^T tiles = K@Q^T; exp on ACT (scale=1/8 folded in); causal diagonal via one
tril tensor_mul per diag k-tile + width-restricted PV; ones column of V_aug
gives the softmax denominator in PSUM row 64; normalize with DVE
reciprocal_approx_fast + gpsimd partition_broadcast + DVE multiply into the
resident ctx^T tile; output projection at the end.

v3: bf16 datapath (tolerance is 2e-2; halves DMA bytes, LDWEIGHTS stalls,
SBUF traffic; PE matmul rate is 1 cycle/row either way). Causal-restricted
S^T matmul + exp widths on diagonal k-tiles. Wo prefetched right after the
prologue.

v4 (final, ~332us vs 423us baseline): space heater removed (the pipeline
is dense enough that DVFS stays up on real work). Output projection
dribbled into hp3's attention via OutEmitter, leaving only j=3's four
l-tiles for the drain. hp0 runs its q-chunks descending so j=3's 32 S
matmuls hide the cold-ACT exp latency at the prologue boundary. Normalize
copies ctx out of PSUM immediately (pctx bank freed after ~1us instead of
the full recip/broadcast chain, unblocking the next chunk's first PV).
DMA traffic split across queues: bulk weight loads on the ACT hwdge
queue, x slices alternating sync/ACT, ost stores via gpsimd software DGE
- the gpsimd partition_broadcast in the softmax normalize rides sync-queue
DIRECT2D slots and must not sit behind bulk transfers.
"""

import numpy as np
from contextlib import ExitStack

import concourse.bass as bass
import concourse.tile as tile
from concourse import bacc, mybir
from concourse.masks import make_identity

F32 = mybir.dt.float32
F32R = mybir.dt.float32r
BF16 = mybir.dt.bfloat16
AF = mybir.ActivationFunctionType

B, L, D = 4, 2048, 1024
N_HEAD, DH, DV = 16, 64, 64
N_CORES = 8
HPC = N_HEAD // 2          # heads per core (8)
OC = HPC * DH              # per-core projection width (512)
NHP = HPC // 2             # head-pairs per core (4)


class ProjEmitter:
    """Q^T/K^T projection for one head-pair, emitted in per-(proj,chunk)
    units so the matmuls interleave with attention of the previous pair."""

    def __init__(self, nc, hp, pools, xt, wq, wk, nch):
        self.nc = nc
        self.xt = xt
        qkp, wp, self.psP = pools
        self.wq_sb = wp.tile([128, 8, 128], BF16, tag="wq")
        self.wk_sb = wp.tile([128, 8, 128], BF16, tag="wk")
        # ACT hwdge queue: keeps bulk weight loads off the sync queue,
        # whose DIRECT2D slots the normalize partition_broadcasts ride
        nc.scalar.dma_start(
            out=self.wq_sb,
            in_=wq[:, hp * 128:(hp + 1) * 128].rearrange("(t p) o -> p t o", p=128))
        nc.scalar.dma_start(
            out=self.wk_sb,
            in_=wk[:, hp * 128:(hp + 1) * 128].rearrange("(t p) o -> p t o", p=128))
        self.qt = qkp.tile([128, nch * 512], BF16, tag="qt")
        self.kt = qkp.tile([128, nch * 512], BF16, tag="kt")
        self.units = [(w, d, c) for w, d in ((self.wq_sb, self.qt),
                                             (self.wk_sb, self.kt))
                      for c in range(nch)]
        self.i = 0

    def step(self):
        if self.i >= len(self.units):
            return False
        w_sb, dst, c = self.units[self.i]
        self.i += 1
        nc = self.nc
        pp = self.psP.tile([128, 512], F32, tag="pp")
        for d in range(8):
            nc.tensor.matmul(pp, w_sb[:, d, :],
                             self.xt[:, d, c * 512:(c + 1) * 512],
                             start=(d == 0), stop=(d == 7))
        nc.vector.tensor_copy(dst[:, c * 512:(c + 1) * 512], pp)
        return True

    def drain(self):
        while self.step():
            pass


class OutEmitter:
    """Output projection, dribbled into hp3's attention: after head-pair 3
    finishes q-chunk j, the four l-tiles 4j..4j+3 have all heads' ctx ready
    and their out-projection can interleave with the remaining attention."""

    def __init__(self, nc, pools, ct, wo_sb, out):
        self.nc = nc
        self.phco, self.psP = pools
        self.ct, self.wo_sb, self.out = ct, wo_sb, out
        self.queue = []
        self.ost = {}
        self.pa = {}

    def add_lt(self, lt):
        self.queue.extend([("full", lt, 0), ("full", lt, 1)])

    def add_partial(self, lt):
        # v=0..2 contraction only: these l-tiles' head-pair 0-2 ctx is
        # ready before hp3's attention even starts, so 3/4 of the last
        # tiles' projection work dribbles instead of sitting in the drain
        self.queue.extend([("pA", lt, 0), ("pA", lt, 1)])

    def add_finish(self, lt):
        self.queue.extend([("fin", lt, 0), ("fin", lt, 1)])

    def step(self):
        if not self.queue:
            return False
        kind, lt, n = self.queue.pop(0)
        nc = self.nc
        if kind == "pA":
            pp = self.psP.tile([128, 512], F32, tag="pp", name="ppo")
            for v in range(3):
                nc.tensor.matmul(pp, self.ct[:, v, lt * 128:(lt + 1) * 128],
                                 self.wo_sb[:, v, n * 512:(n + 1) * 512],
                                 start=(v == 0), stop=(v == 2))
            pa = self.phco.tile([128, 512], F32, tag="pA", bufs=8,
                                name=f"pA{lt}_{n}")
            nc.vector.tensor_copy(pa, pp)
            self.pa[(lt, n)] = pa
            return True
        if n == 0:
            self.ost[lt] = self.phco.tile([128, D], F32, tag="ost",
                                          name=f"ost{lt}")
        ost = self.ost[lt]
        pp = self.psP.tile([128, 512], F32, tag="pp", name="ppo")
        if kind == "fin":
            nc.tensor.matmul(pp, self.ct[:, 3, lt * 128:(lt + 1) * 128],
                             self.wo_sb[:, 3, n * 512:(n + 1) * 512],
                             start=True, stop=True)
            nc.vector.tensor_add(ost[:, n * 512:(n + 1) * 512], pp,
                                 self.pa.pop((lt, n)))
        else:
            for v in range(4):
                nc.tensor.matmul(pp, self.ct[:, v, lt * 128:(lt + 1) * 128],
                                 self.wo_sb[:, v, n * 512:(n + 1) * 512],
                                 start=(v == 0), stop=(v == 3))
            nc.vector.tensor_copy(ost[:, n * 512:(n + 1) * 512], pp)
        if n == 1:
            # gpsimd software DGE: ost stores would otherwise queue ahead
            # of hp3's normalize broadcasts on the sync queue
            nc.gpsimd.dma_start(out=self.out[lt * 128:(lt + 1) * 128, :],
                                in_=ost)
            del self.ost[lt]
        return True

    def drain(self):
        while self.step():
            pass


def build_nc(l=L):
    assert l % 512 == 0
    nch = l // 512           # q-chunks
    nlt = l // 128           # l-tiles
    nc = bacc.Bacc("TRN2", target_bir_lowering=False, debug=False,
                   num_devices=N_CORES)

    x = nc.dram_tensor("x", [l, D], BF16, kind="ExternalInput").ap()
    wq = nc.dram_tensor("wq", [D, OC], BF16, kind="ExternalInput").ap()
    wk = nc.dram_tensor("wk", [D, OC], BF16, kind="ExternalInput").ap()
    wv = nc.dram_tensor("wv", [D, OC], BF16, kind="ExternalInput").ap()
    wo = nc.dram_tensor("wo", [OC, D], BF16, kind="ExternalInput").ap()
    out = nc.dram_tensor("out", [l, D], F32, kind="ExternalOutput").ap()

    with tile.TileContext(nc) as tc, ExitStack() as ctx:
        top = ctx.enter_context(tc.tile_pool(name="top", bufs=1))
        xtp = ctx.enter_context(tc.tile_pool(name="xtp", bufs=1))
        qkp = ctx.enter_context(tc.tile_pool(name="qkp", bufs=2))
        wp = ctx.enter_context(tc.tile_pool(name="wp", bufs=2))
        phco = ctx.enter_context(tc.tile_pool(name="phco", bufs=3))

        # V: [128(l), ltile, head, 65] - col 64 is ones (softmax denominator)
        vt = top.tile([128, nlt, HPC, DH + 1], BF16)
        ct = top.tile([128, NHP, l], BF16)        # normalized ctx^T
        tril = top.tile([128, 128], BF16)
        ones = top.tile([128, 1], F32)
        warm = top.tile([128, 1], BF16)
        onesw = top.tile([1, DV], BF16)           # rank-1 bcast weights
        xt = xtp.tile([128, 8, l], BF16)          # x^T, d-major

        nc.vector.memset(ones, 1.0)
        nc.vector.memset(onesw, 1.0)
        # warm-up exp: loads the ACT function table during the DMA-bound
        # startup instead of stalling the first attention group
        nc.scalar.activation(warm, ones, AF.Exp, scale=0.125)
        nc.vector.tensor_copy(
            vt[:, :, :, DV:DV + 1].rearrange("p t h c -> p (t h) c"),
            ones.broadcast_to((128, nlt * HPC, 1)))
        # causal keep-mask for S^T diag blocks: tril[k, q] = 1.0 iff q >= k
        nc.gpsimd.memset(tril, 0.0)
        nc.gpsimd.affine_select(
            out=tril, in_=tril, compare_op=mybir.AluOpType.is_gt,
            fill=1.0, base=0, pattern=[[-1, 128]], channel_multiplier=1)

        # ---------------- Prologue: transpose + V + QK(hp=0) --------------
        with tc.tile_pool(name="pro", bufs=6) as pro, \
             tc.tile_pool(name="wvp", bufs=1) as wvp, \
             tc.tile_pool(name="psPro", bufs=2, space="PSUM") as psPro, \
             tc.tile_pool(name="psT", bufs=4, space="PSUM") as psT:
            ident = wvp.tile([128, 128], BF16)
            make_identity(nc, ident)
            # chunk-0 x slices first so the first transposes start ASAP;
            # only then the Wv load (needed one chunk later)
            # Wv leads the scalar queue (V(c0) waits on it) while chunk 0's
            # x slices stream on sync; later chunks alternate both queues
            wv_sb = wvp.tile([128, 8, OC], BF16)
            nc.scalar.dma_start(out=wv_sb,
                                in_=wv.rearrange("(t p) o -> p t o", p=128))
            xst0 = []
            for s in range(4):
                xst = pro.tile([128, D], BF16, tag="xst", name=f"xst0{s}")
                nc.sync.dma_start(out=xst, in_=x[s * 128:(s + 1) * 128, :])
                xst0.append(xst)
            for c in range(nch):
                for s in range(4):
                    if c == 0:
                        xst = xst0[s]
                    else:
                        xst = pro.tile([128, D], BF16, tag="xst")
                        eng = nc.sync if s % 2 == 0 else nc.scalar
                        eng.dma_start(
                            out=xst,
                            in_=x[c * 512 + s * 128: c * 512 + (s + 1) * 128, :])
                    # all 8 d-blocks transpose into one 1-bank PSUM tile,
                    # drained by a single wide DVE copy (8x fewer copies)
                    pt8 = psT.tile([128, 8, 128], BF16, tag="pt")
                    for d in range(8):
                        nc.tensor.transpose(
                            pt8[:, d, :], xst[:, d * 128:(d + 1) * 128],
                            ident)
                    nc.vector.tensor_copy(
                        xt[:, :, c * 512 + s * 128: c * 512 + (s + 1) * 128],
                        pt8)
                # V for this l-chunk
                for m in range(4):
                    pp = psPro.tile([128, 512], F32, tag="pp")
                    for d in range(8):
                        nc.tensor.matmul(
                            pp, xt[:, d, (c * 4 + m) * 128:(c * 4 + m + 1) * 128],
                            wv_sb[:, d, :], start=(d == 0), stop=(d == 7))
                    nc.vector.tensor_copy(
                        vt[:, c * 4 + m, :, 0:DV],
                        pp.rearrange("p (h v) -> p h v", h=HPC))
                if c == 1:
                    # issue hp0's Wq/Wk DMAs mid-prologue so the QK drain
                    # at the end doesn't stall on them
                    em = ProjEmitter(nc, 0, (qkp, wp, psPro), xt, wq, wk, nch)
            em.drain()

        # Prefetch Wo now: the DMA queue is idle during attention, and the
        # output projection otherwise stalls ~14us on this load at the end.
        phc = ctx.enter_context(tc.tile_pool(name="phc", bufs=1))
        wo_sb = phc.tile([128, 4, D], BF16)
        nc.scalar.dma_start(out=wo_sb,
                            in_=wo.rearrange("(t p) o -> p t o", p=128))

        # ---------------- Main: attention + next-pair projections ---------
        with tc.tile_pool(name="phb", bufs=2) as phb, \
             tc.tile_pool(name="psS", bufs=2, space="PSUM") as psS, \
             tc.tile_pool(name="psPd", bufs=2, space="PSUM") as psPd, \
             tc.tile_pool(name="psC", bufs=2, space="PSUM") as psC:
            n_groups_hp = 2 * nch * (nch + 1)
            for hp in range(NHP):
                qt, kt = em.qt, em.kt
                if hp + 1 < NHP:
                    em = ProjEmitter(nc, hp + 1, (qkp, wp, psPd), xt, wq, wk,
                                     nch)
                    cadence = max(1, (n_groups_hp // 2) // (2 * nch))
                else:
                    em = OutEmitter(nc, (phco, psPd), ct, wo_sb, out)
                    cadence = 1
                gcount = 0

                def pv_step(g, j, pctx, pexp, po, H):
                    # masks + PV for group g (one group after its exp)
                    for r2 in range(2):
                        kt_i = 2 * g + r2
                        r = kt_i - 4 * j
                        c0 = 0
                        if r >= 0:      # diagonal k-tile
                            c0 = r * 128
                            nc.vector.tensor_mul(
                                pexp[:, r2, c0:c0 + 128],
                                pexp[:, r2, c0:c0 + 128], tril)
                        nc.tensor.matmul(
                            pctx[:, c0:512],
                            vt[:, kt_i, H, :],
                            pexp[:, r2, c0:512],
                            start=(kt_i == 0), stop=(kt_i == 4 * j + 3))

                # both heads interleaved at group level: two independent
                # dependency chains keep the PE busy through each other's
                # exp waits. hp0 runs j descending: at the prologue
                # boundary the ACT pipeline is cold, and j=3 offers 32 S
                # matmuls to hide the first exp latencies (j=0 only 8).
                jorder = range(nch - 1, -1, -1) if hp == 0 else range(nch)
                for j in jorder:
                    if hp == NHP - 1:
                        # final j: spread the remaining dribble units so
                        # some out-proj work still sits in the PE queue
                        # while the last normalize chain runs
                        cadence = 2 if j == nch - 1 else 1
                    n_g = 2 * (j + 1)
                    pctxs = {}
                    prevs = {0: None, 1: None}
                    for h in range(2):
                        pctxs[h] = psC.tile([DV + 1, 512], F32,
                                            tag="pctx", name=f"pctx{h}")
                    for g in range(n_g + 1):
                        for h in range(2):
                            po = 64 * h
                            H = 2 * hp + h
                            pexp = None
                            if g < n_g:
                                psc = psS.tile([128, 2, 512], F32,
                                               tag="psc", name=f"psc{h}")
                                for r2 in range(2):
                                    kt_i = 2 * g + r2
                                    # causal: diag k-tile kt_i only feeds
                                    # q columns >= (kt_i-4j)*128
                                    c0 = max(0, kt_i - 4 * j) * 128
                                    nc.tensor.matmul(
                                        psc[:, r2, c0:512],
                                        kt[po:po + DH,
                                           kt_i * 128:(kt_i + 1) * 128],
                                        qt[po:po + DH,
                                           j * 512 + c0:(j + 1) * 512],
                                        start=True, stop=True)
                                pexp = phb.tile([128, 2, 512], BF16,
                                                tag="pexp", bufs=10,
                                                name=f"pexp{h}")
                                cg = max(0, 2 * g - 4 * j) * 128
                                nc.scalar.activation(pexp[:, :, cg:512],
                                                     psc[:, :, cg:512],
                                                     AF.Exp, scale=0.125)
                            if prevs[h] is not None:
                                pv_step(prevs[h][0], j, pctxs[h],
                                        prevs[h][1], po, H)
                                gcount += 1
                                if em and cadence and gcount % cadence == 0:
                                    em.step()
                            prevs[h] = (g, pexp) if g < n_g else None
                    for h in range(2):
                        po = 64 * h
                        # free the pctx PSUM bank after two quick DVE copies
                        # (the next j's first PV waits on it); the
                        # recip/broadcast/scale dangle off the critical path.
                        # ctu is a base-0 staging tile so the scale-mul's two
                        # SBUF inputs share a base partition.
                        rs = phb.tile([1, 512], F32, tag="rs", name="rs")
                        nc.vector.tensor_copy(rs, pctxs[h][DV:DV + 1, :])
                        ctu = phb.tile([64, 512], BF16, tag="ctu",
                                       name=f"ctu{h}")
                        nc.vector.tensor_copy(ctu, pctxs[h][0:DV, :])
                        inv = phb.tile([1, 512], F32, tag="inv", name="inv")
                        nc.vector.reciprocal_approx_fast(out=inv, in_=rs)
                        if hp == NHP - 1 and j == nch - 1:
                            # last chunk: the out-proj drain waits on this
                            # chain, so broadcast via a rank-1 PE matmul
                            # (~0.2us, PE is idling here) instead of the
                            # ~1us gpsimd DIRECT2D path
                            invb = phb.tile([1, 512], BF16, tag="invb",
                                            name="invb")
                            nc.vector.tensor_copy(invb, inv)
                            bcp = psPd.tile([128, 512], F32, tag="pp",
                                            name="bcp")
                            nc.tensor.matmul(bcp[0:DV, :], onesw, invb,
                                             start=True, stop=True)
                            nc.vector.tensor_mul(
                                ct[po:po + DV, hp, j * 512:(j + 1) * 512],
                                ctu, bcp[0:DV, :])
                        else:
                            bc = phb.tile([64, 512], F32, tag="bc",
                                          name="bc")
                            nc.gpsimd.partition_broadcast(out_ap=bc,
                                                          in_ap=inv)
                            nc.vector.tensor_mul(
                                ct[po:po + DV, hp, j * 512:(j + 1) * 512],
                                ctu, bc)
                    if hp == NHP - 1:
                        # all heads' ctx for l-tiles 4j..4j+3 now ready;
                        # queue their output projection for dribbling
                        for lt in range(4 * j, 4 * j + 4):
                            em.add_lt(lt)
                em.drain()

    nc.compile()
    return nc


def _bf16(a):
    import ml_dtypes
    return np.ascontiguousarray(a).astype(ml_dtypes.bfloat16)


def make_in_maps(x, Wq, Wk, Wv, Wo):
    in_maps = []
    for c in range(N_CORES):
        b, g = c // 2, c % 2
        in_maps.append({
            "x": _bf16(x[b]),
            "wq": _bf16(Wq[:, g * OC:(g + 1) * OC]),
            "wk": _bf16(Wk[:, g * OC:(g + 1) * OC]),
            "wv": _bf16(Wv[:, g * OC:(g + 1) * OC]),
            "wo": _bf16(Wo[g * OC:(g + 1) * OC, :]),
        })
    return in_maps


_NC_CACHE = {}


def _get_nc():
    if "nc" not in _NC_CACHE:
        _NC_CACHE["nc"] = build_nc()
    return _NC_CACHE["nc"]


def _numpy_fallback(x, Wq, Wk, Wv, Wo, bo, mask):
    Bsz, Lq, _ = x.shape
    Q = (x @ Wq).reshape(Bsz, Lq, N_HEAD, DH).transpose(0, 2, 1, 3)
    K = (x @ Wk).reshape(Bsz, Lq, N_HEAD, DH).transpose(0, 2, 1, 3)
    V = (x @ Wv).reshape(Bsz, Lq, N_HEAD, DV).transpose(0, 2, 1, 3)
    s = np.einsum("bhqd,bhkd->bhqk", Q, K) / np.sqrt(np.float32(DH))
    s = np.where(mask, s, -np.inf)
    s = s - s.max(axis=-1, keepdims=True)
    p = np.exp(s)
    p /= p.sum(axis=-1, keepdims=True)
    ctxv = np.einsum("bhqk,bhkv->bhqv", p, V)
    ctxv = ctxv.transpose(0, 2, 1, 3).reshape(Bsz, Lq, N_HEAD * DV)
    return (ctxv @ Wo + bo).astype(np.float32)


def run_on_hw(in_maps, trace=False):
    from concourse.bass_utils import run_bass_kernel_spmd
    nc = _get_nc()
    return run_bass_kernel_spmd(nc, in_maps, list(range(N_CORES)), trace=trace)


def kernel(x, Wq, Wk, Wv, Wo, bo, mask, _trace=False, _results=None):
    x = np.asarray(x, dtype=np.float32)
    Wq = np.asarray(Wq, dtype=np.float32)
    Wk = np.asarray(Wk, dtype=np.float32)
    Wv = np.asarray(Wv, dtype=np.float32)
    Wo = np.asarray(Wo, dtype=np.float32)
    bo = np.asarray(bo, dtype=np.float32)
    mask_np = np.asarray(mask).reshape(mask.shape[-2], mask.shape[-1])

    causal = bool(np.array_equal(
        mask_np, np.tril(np.ones((L, L), dtype=bool))))
    if not causal or x.shape != (B, L, D):
        return _numpy_fallback(np.asarray(x), Wq, Wk, Wv, Wo, bo,
                               np.asarray(mask))

    res = run_on_hw(make_in_maps(x, Wq, Wk, Wv, Wo), trace=_trace)
    if _results is not None:
        _results.append(res)
    out = np.empty((B, L, D), dtype=np.float32)
    for b in range(B):
        out[b] = (np.asarray(res.results[2 * b]["out"], dtype=np.float32)
                  + np.asarray(res.results[2 * b + 1]["out"], dtype=np.float32)
                  + bo)
    return out



# revision 73
# speedup vs baseline: 1.2198x; 1.0023x over previous
"""Causal multi-head attention Trainium2 kernel (8 NeuronCores).

Problem: B=4, L=2048, D=1024, 16 heads x (dh=64, dv=64), causal mask.
Sharding: data-parallel over batch (4) x tensor-parallel over heads (2 groups
of 8). Core c handles batch c//2, head-group c%2. Each core computes its
partial output projection (ctx_g @ Wo_g); the host sums the two head-group
partials per batch and adds the bias.

v2: software-pipelined. The prologue transposes x (PE) into a resident
xT [d, l] tile, computes V = x@Wv (+ones column for the fused softmax
denominator) and Q^T/K^T for head-pair 0. The main loop runs flash-style
attention per head-pair while dribbling the next head-pair's Q^T/K^T
projection matmuls between attention groups — keeping the PE dense so the
HAM clock gate stays at 2.4 GHz.
S^T tiles = K@Q^T; exp on ACT (scale=1/8 folded in); causal diagonal via one
tril tensor_mul per diag k-tile + width-restricted PV; ones column of V_aug
gives the softmax denominator in PSUM row 64; normalize with DVE
reciprocal_approx_fast + gpsimd partition_broadcast + DVE multiply into the
resident ctx^T tile; output projection at the end.

v3: bf16 datapath (tolerance is 2e-2; halves DMA bytes, LDWEIGHTS stalls,
SBUF traffic; PE matmul rate is 1 cycle/row either way). Causal-restricted
S^T matmul + exp widths on diagonal k-tiles. Wo prefetched right after the
prologue.

v4 (final, ~332us vs 423us baseline): space heater removed (the pipeline
is dense enough that DVFS stays up on real work). Output projection
dribbled into hp3's attention via OutEmitter, leaving only j=3's four
l-tiles for the drain. hp0 runs its q-chunks descending so j=3's 32 S
matmuls hide the cold-ACT exp latency at the prologue boundary. Normalize
copies ctx out of PSUM immediately (pctx bank freed after ~1us instead of
the full recip/broadcast chain, unblocking the next chunk's first PV).
DMA traffic split across queues: bulk weight loads on the ACT hwdge
queue, x slices alternating sync/ACT, ost stores via gpsimd software DGE
- the gpsimd partition_broadcast in the softmax normalize rides sync-queue
DIRECT2D slots and must not sit behind bulk transfers.
"""

import numpy as np
from contextlib import ExitStack

import concourse.bass as bass
import concourse.tile as tile
from concourse import bacc, mybir
from concourse.masks import make_identity

F32 = mybir.dt.float32
F32R = mybir.dt.float32r
BF16 = mybir.dt.bfloat16
AF = mybir.ActivationFunctionType

B, L, D = 4, 2048, 1024
N_HEAD, DH, DV = 16, 64, 64
N_CORES = 8
HPC = N_HEAD // 2          # heads per core (8)
OC = HPC * DH              # per-core projection width (512)
NHP = HPC // 2             # head-pairs per core (4)


class ProjEmitter:
    """Q^T/K^T projection for one head-pair, emitted in per-(proj,chunk)
    units so the matmuls interleave with attention of the previous pair."""

    def __init__(self, nc, hp, pools, xt, wq, wk, nch):
        self.nc = nc
        self.xt = xt
        qkp, wp, self.psP = pools
        self.wq_sb = wp.tile([128, 8, 128], BF16, tag="wq")
        self.wk_sb = wp.tile([128, 8, 128], BF16, tag="wk")
        # ACT hwdge queue: keeps bulk weight loads off the sync queue,
        # whose DIRECT2D slots the normalize partition_broadcasts ride
        nc.scalar.dma_start(
            out=self.wq_sb,
            in_=wq[:, hp * 128:(hp + 1) * 128].rearrange("(t p) o -> p t o", p=128))
        nc.scalar.dma_start(
            out=self.wk_sb,
            in_=wk[:, hp * 128:(hp + 1) * 128].rearrange("(t p) o -> p t o", p=128))
        self.qt = qkp.tile([128, nch * 512], BF16, tag="qt")
        self.kt = qkp.tile([128, nch * 512], BF16, tag="kt")
        self.units = [(w, d, c) for w, d in ((self.wq_sb, self.qt),
                                             (self.wk_sb, self.kt))
                      for c in range(nch)]
        self.i = 0

    def step(self):
        if self.i >= len(self.units):
            return False
        w_sb, dst, c = self.units[self.i]
        self.i += 1
        nc = self.nc
        pp = self.psP.tile([128, 512], F32, tag="pp")
        for d in range(8):
            nc.tensor.matmul(pp, w_sb[:, d, :],
                             self.xt[:, d, c * 512:(c + 1) * 512],
                             start=(d == 0), stop=(d == 7))
        nc.vector.tensor_copy(dst[:, c * 512:(c + 1) * 512], pp)
        return True

    def drain(self):
        while self.step():
            pass


class OutEmitter:
    """Output projection, dribbled into hp3's attention: after head-pair 3
    finishes q-chunk j, the four l-tiles 4j..4j+3 have all heads' ctx ready
    and their out-projection can interleave with the remaining attention."""

    def __init__(self, nc, pools, ct, wo_sb, out):
        self.nc = nc
        self.phco, self.psP = pools
        self.ct, self.wo_sb, self.out = ct, wo_sb, out
        self.queue = []
        self.ost = {}
        self.pa = {}

    def add_lt(self, lt):
        self.queue.extend([("full", lt, 0), ("full", lt, 1)])

    def add_partial(self, lt):
        # v=0..2 contraction only: these l-tiles' head-pair 0-2 ctx is
        # ready before hp3's attention even starts, so 3/4 of the last
        # tiles' projection work dribbles instead of sitting in the drain
        self.queue.extend([("pA", lt, 0), ("pA", lt, 1)])

    def add_finish(self, lt):
        self.queue.extend([("fin", lt, 0), ("fin", lt, 1)])

    def step(self):
        if not self.queue:
            return False
        kind, lt, n = self.queue.pop(0)
        nc = self.nc
        if kind == "pA":
            pp = self.psP.tile([128, 512], F32, tag="pp", name="ppo")
            for v in range(3):
                nc.tensor.matmul(pp, self.ct[:, v, lt * 128:(lt + 1) * 128],
                                 self.wo_sb[:, v, n * 512:(n + 1) * 512],
                                 start=(v == 0), stop=(v == 2))
            pa = self.phco.tile([128, 512], F32, tag="pA", bufs=8,
                                name=f"pA{lt}_{n}")
            nc.vector.tensor_copy(pa, pp)
            self.pa[(lt, n)] = pa
            return True
        if n == 0:
            self.ost[lt] = self.phco.tile([128, D], F32, tag="ost",
                                          name=f"ost{lt}")
        ost = self.ost[lt]
        pp = self.psP.tile([128, 512], F32, tag="pp", name="ppo")
        if kind == "fin":
            nc.tensor.matmul(pp, self.ct[:, 3, lt * 128:(lt + 1) * 128],
                             self.wo_sb[:, 3, n * 512:(n + 1) * 512],
                             start=True, stop=True)
            nc.vector.tensor_add(ost[:, n * 512:(n + 1) * 512], pp,
                                 self.pa.pop((lt, n)))
        else:
            for v in range(4):
                nc.tensor.matmul(pp, self.ct[:, v, lt * 128:(lt + 1) * 128],
                                 self.wo_sb[:, v, n * 512:(n + 1) * 512],
                                 start=(v == 0), stop=(v == 3))
            nc.vector.tensor_copy(ost[:, n * 512:(n + 1) * 512], pp)
        if lt >= 12:
            # drain-phase tiles: no normalize broadcasts remain on the
            # sync queue (the last chunk uses the PE broadcast), so use
            # the fast hw DGE there, per-half, and skip the slow gpsimd
            # DGE drain on the critical tail
            nc.sync.dma_start(
                out=self.out[lt * 128:(lt + 1) * 128,
                             n * 512:(n + 1) * 512],
                in_=ost[:, n * 512:(n + 1) * 512])
            if n == 1:
                del self.ost[lt]
        elif n == 1:
            # gpsimd software DGE: ost stores would otherwise queue ahead
            # of hp3's normalize broadcasts on the sync queue
            nc.gpsimd.dma_start(out=self.out[lt * 128:(lt + 1) * 128, :],
                                in_=ost)
            del self.ost[lt]
        return True

    def drain(self):
        while self.step():
            pass


def build_nc(l=L):
    assert l % 512 == 0
    nch = l // 512           # q-chunks
    nlt = l // 128           # l-tiles
    nc = bacc.Bacc("TRN2", target_bir_lowering=False, debug=False,
                   num_devices=N_CORES)

    x = nc.dram_tensor("x", [l, D], BF16, kind="ExternalInput").ap()
    wq = nc.dram_tensor("wq", [D, OC], BF16, kind="ExternalInput").ap()
    wk = nc.dram_tensor("wk", [D, OC], BF16, kind="ExternalInput").ap()
    wv = nc.dram_tensor("wv", [D, OC], BF16, kind="ExternalInput").ap()
    wo = nc.dram_tensor("wo", [OC, D], BF16, kind="ExternalInput").ap()
    out = nc.dram_tensor("out", [l, D], F32, kind="ExternalOutput").ap()

    with tile.TileContext(nc) as tc, ExitStack() as ctx:
        top = ctx.enter_context(tc.tile_pool(name="top", bufs=1))
        xtp = ctx.enter_context(tc.tile_pool(name="xtp", bufs=1))
        qkp = ctx.enter_context(tc.tile_pool(name="qkp", bufs=2))
        wp = ctx.enter_context(tc.tile_pool(name="wp", bufs=2))
        phco = ctx.enter_context(tc.tile_pool(name="phco", bufs=3))

        # V: [128(l), ltile, head, 65] - col 64 is ones (softmax denominator)
        vt = top.tile([128, nlt, HPC, DH + 1], BF16)
        ct = top.tile([128, NHP, l], BF16)        # normalized ctx^T
        tril = top.tile([128, 128], BF16)
        ones = top.tile([128, 1], F32)
        warm = top.tile([128, 1], BF16)
        onesw = top.tile([1, DV], BF16)           # rank-1 bcast weights
        xt = xtp.tile([128, 8, l], BF16)          # x^T, d-major

        nc.vector.memset(ones, 1.0)
        nc.vector.memset(onesw, 1.0)
        # warm-up exp: loads the ACT function table during the DMA-bound
        # startup instead of stalling the first attention group
        nc.scalar.activation(warm, ones, AF.Exp, scale=0.125)
        nc.vector.tensor_copy(
            vt[:, :, :, DV:DV + 1].rearrange("p t h c -> p (t h) c"),
            ones.broadcast_to((128, nlt * HPC, 1)))
        # causal keep-mask for S^T diag blocks: tril[k, q] = 1.0 iff q >= k
        nc.gpsimd.memset(tril, 0.0)
        nc.gpsimd.affine_select(
            out=tril, in_=tril, compare_op=mybir.AluOpType.is_gt,
            fill=1.0, base=0, pattern=[[-1, 128]], channel_multiplier=1)

        # ---------------- Prologue: transpose + V + QK(hp=0) --------------
        with tc.tile_pool(name="pro", bufs=6) as pro, \
             tc.tile_pool(name="wvp", bufs=1) as wvp, \
             tc.tile_pool(name="psPro", bufs=2, space="PSUM") as psPro, \
             tc.tile_pool(name="psT", bufs=4, space="PSUM") as psT:
            ident = wvp.tile([128, 128], BF16)
            make_identity(nc, ident)
            # chunk-0 x slices first so the first transposes start ASAP;
            # only then the Wv load (needed one chunk later)
            # Wv leads the scalar queue (V(c0) waits on it) while chunk 0's
            # x slices stream on sync; later chunks alternate both queues
            wv_sb = wvp.tile([128, 8, OC], BF16)
            nc.scalar.dma_start(out=wv_sb,
                                in_=wv.rearrange("(t p) o -> p t o", p=128))
            xst0 = []
            for s in range(4):
                xst = pro.tile([128, D], BF16, tag="xst", name=f"xst0{s}")
                nc.sync.dma_start(out=xst, in_=x[s * 128:(s + 1) * 128, :])
                xst0.append(xst)
            for c in range(nch):
                for s in range(4):
                    if c == 0:
                        xst = xst0[s]
                    else:
                        xst = pro.tile([128, D], BF16, tag="xst")
                        eng = nc.sync if s % 2 == 0 else nc.scalar
                        eng.dma_start(
                            out=xst,
                            in_=x[c * 512 + s * 128: c * 512 + (s + 1) * 128, :])
                    # all 8 d-blocks transpose into one 1-bank PSUM tile,
                    # drained by a single wide DVE copy (8x fewer copies)
                    pt8 = psT.tile([128, 8, 128], BF16, tag="pt")
                    for d in range(8):
                        nc.tensor.transpose(
                            pt8[:, d, :], xst[:, d * 128:(d + 1) * 128],
                            ident)
                    nc.vector.tensor_copy(
                        xt[:, :, c * 512 + s * 128: c * 512 + (s + 1) * 128],
                        pt8)
                # V for this l-chunk
                for m in range(4):
                    pp = psPro.tile([128, 512], F32, tag="pp")
                    for d in range(8):
                        nc.tensor.matmul(
                            pp, xt[:, d, (c * 4 + m) * 128:(c * 4 + m + 1) * 128],
                            wv_sb[:, d, :], start=(d == 0), stop=(d == 7))
                    nc.vector.tensor_copy(
                        vt[:, c * 4 + m, :, 0:DV],
                        pp.rearrange("p (h v) -> p h v", h=HPC))
                if c == 1:
                    # issue hp0's Wq/Wk DMAs mid-prologue so the QK drain
                    # at the end doesn't stall on them
                    em = ProjEmitter(nc, 0, (qkp, wp, psPro), xt, wq, wk, nch)
            em.drain()

        # Prefetch Wo now: the DMA queue is idle during attention, and the
        # output projection otherwise stalls ~14us on this load at the end.
        phc = ctx.enter_context(tc.tile_pool(name="phc", bufs=1))
        wo_sb = phc.tile([128, 4, D], BF16)
        nc.scalar.dma_start(out=wo_sb,
                            in_=wo.rearrange("(t p) o -> p t o", p=128))

        # ---------------- Main: attention + next-pair projections ---------
        with tc.tile_pool(name="phb", bufs=2) as phb, \
             tc.tile_pool(name="psS", bufs=2, space="PSUM") as psS, \
             tc.tile_pool(name="psPd", bufs=2, space="PSUM") as psPd, \
             tc.tile_pool(name="psC", bufs=2, space="PSUM") as psC:
            n_groups_hp = 2 * nch * (nch + 1)
            for hp in range(NHP):
                qt, kt = em.qt, em.kt
                if hp + 1 < NHP:
                    em = ProjEmitter(nc, hp + 1, (qkp, wp, psPd), xt, wq, wk,
                                     nch)
                    cadence = max(1, (n_groups_hp // 2) // (2 * nch))
                else:
                    em = OutEmitter(nc, (phco, psPd), ct, wo_sb, out)
                    cadence = 1
                gcount = 0

                def pv_step(g, j, pctx, pexp, po, H):
                    # masks + PV for group g (one group after its exp)
                    for r2 in range(2):
                        kt_i = 2 * g + r2
                        r = kt_i - 4 * j
                        c0 = 0
                        if r >= 0:      # diagonal k-tile
                            c0 = r * 128
                            nc.vector.tensor_mul(
                                pexp[:, r2, c0:c0 + 128],
                                pexp[:, r2, c0:c0 + 128], tril)
                        nc.tensor.matmul(
                            pctx[:, c0:512],
                            vt[:, kt_i, H, :],
                            pexp[:, r2, c0:512],
                            start=(kt_i == 0), stop=(kt_i == 4 * j + 3))

                # both heads interleaved at group level: two independent
                # dependency chains keep the PE busy through each other's
                # exp waits. hp0 runs j descending: at the prologue
                # boundary the ACT pipeline is cold, and j=3 offers 32 S
                # matmuls to hide the first exp latencies (j=0 only 8).
                jorder = range(nch - 1, -1, -1) if hp == 0 else range(nch)
                for j in jorder:
                    if hp == NHP - 1:
                        # final j: spread the remaining dribble units so
                        # some out-proj work still sits in the PE queue
                        # while the last normalize chain runs
                        cadence = 2 if j == nch - 1 else 1
                    n_g = 2 * (j + 1)
                    pctxs = {}
                    prevs = {0: None, 1: None}
                    for h in range(2):
                        pctxs[h] = psC.tile([DV + 1, 512], F32,
                                            tag="pctx", name=f"pctx{h}")
                    for g in range(n_g + 1):
                        for h in range(2):
                            po = 64 * h
                            H = 2 * hp + h
                            pexp = None
                            if g < n_g:
                                psc = psS.tile([128, 2, 512], F32,
                                               tag="psc", name=f"psc{h}")
                                for r2 in range(2):
                                    kt_i = 2 * g + r2
                                    # causal: diag k-tile kt_i only feeds
                                    # q columns >= (kt_i-4j)*128
                                    c0 = max(0, kt_i - 4 * j) * 128
                                    nc.tensor.matmul(
                                        psc[:, r2, c0:512],
                                        kt[po:po + DH,
                                           kt_i * 128:(kt_i + 1) * 128],
                                        qt[po:po + DH,
                                           j * 512 + c0:(j + 1) * 512],
                                        start=True, stop=True)
                                pexp = phb.tile([128, 2, 512], BF16,
                                                tag="pexp", bufs=10,
                                                name=f"pexp{h}")
                                cg = max(0, 2 * g - 4 * j) * 128
                                nc.scalar.activation(pexp[:, :, cg:512],
                                                     psc[:, :, cg:512],
                                                     AF.Exp, scale=0.125)
                            if prevs[h] is not None:
                                pv_step(prevs[h][0], j, pctxs[h],
                                        prevs[h][1], po, H)
                                gcount += 1
                                if em and cadence and gcount % cadence == 0:
                                    em.step()
                            prevs[h] = (g, pexp) if g < n_g else None
                    for h in range(2):
                        po = 64 * h
                        # free the pctx PSUM bank after two quick DVE copies
                        # (the next j's first PV waits on it); the
                        # recip/broadcast/scale dangle off the critical path.
                        # ctu is a base-0 staging tile so the scale-mul's two
                        # SBUF inputs share a base partition.
                        rs = phb.tile([1, 512], F32, tag="rs", name="rs")
                        nc.vector.tensor_copy(rs, pctxs[h][DV:DV + 1, :])
                        ctu = phb.tile([64, 512], BF16, tag="ctu",
                                       name=f"ctu{h}")
                        nc.vector.tensor_copy(ctu, pctxs[h][0:DV, :])
                        inv = phb.tile([1, 512], F32, tag="inv", name="inv")
                        nc.vector.reciprocal_approx_fast(out=inv, in_=rs)
                        if hp == NHP - 1 and j == nch - 1:
                            # last chunk: the out-proj drain waits on this
                            # chain, so broadcast via a rank-1 PE matmul
                            # (~0.2us, PE is idling here) instead of the
                            # ~1us gpsimd DIRECT2D path
                            invb = phb.tile([1, 512], BF16, tag="invb",
                                            name="invb")
                            nc.vector.tensor_copy(invb, inv)
                            bcp = psPd.tile([128, 512], F32, tag="pp",
                                            name="bcp")
                            nc.tensor.matmul(bcp[0:DV, :], onesw, invb,
                                             start=True, stop=True)
                            nc.vector.tensor_mul(
                                ct[po:po + DV, hp, j * 512:(j + 1) * 512],
                                ctu, bcp[0:DV, :])
                        else:
                            bc = phb.tile([64, 512], F32, tag="bc",
                                          name="bc")
                            nc.gpsimd.partition_broadcast(out_ap=bc,
                                                          in_ap=inv)
                            nc.vector.tensor_mul(
                                ct[po:po + DV, hp, j * 512:(j + 1) * 512],
                                ctu, bc)
                    if hp == NHP - 1:
                        # all heads' ctx for l-tiles 4j..4j+3 now ready;
                        # queue their output projection for dribbling
                        for lt in range(4 * j, 4 * j + 4):
                            em.add_lt(lt)
                em.drain()

    nc.compile()
    return nc


def _bf16(a):
    import ml_dtypes
    return np.ascontiguousarray(a).astype(ml_dtypes.bfloat16)


def make_in_maps(x, Wq, Wk, Wv, Wo):
    in_maps = []
    for c in range(N_CORES):
        b, g = c // 2, c % 2
        in_maps.append({
            "x": _bf16(x[b]),
            "wq": _bf16(Wq[:, g * OC:(g + 1) * OC]),
            "wk": _bf16(Wk[:, g * OC:(g + 1) * OC]),
            "wv": _bf16(Wv[:, g * OC:(g + 1) * OC]),
            "wo": _bf16(Wo[g * OC:(g + 1) * OC, :]),
        })
    return in_maps


_NC_CACHE = {}


def _get_nc():
    if "nc" not in _NC_CACHE:
        _NC_CACHE["nc"] = build_nc()
    return _NC_CACHE["nc"]


def _numpy_fallback(x, Wq, Wk, Wv, Wo, bo, mask):
    Bsz, Lq, _ = x.shape
    Q = (x @ Wq).reshape(Bsz, Lq, N_HEAD, DH).transpose(0, 2, 1, 3)
    K = (x @ Wk).reshape(Bsz, Lq, N_HEAD, DH).transpose(0, 2, 1, 3)
    V = (x @ Wv).reshape(Bsz, Lq, N_HEAD, DV).transpose(0, 2, 1, 3)
    s = np.einsum("bhqd,bhkd->bhqk", Q, K) / np.sqrt(np.float32(DH))
    s = np.where(mask, s, -np.inf)
    s = s - s.max(axis=-1, keepdims=True)
    p = np.exp(s)
    p /= p.sum(axis=-1, keepdims=True)
    ctxv = np.einsum("bhqk,bhkv->bhqv", p, V)
    ctxv = ctxv.transpose(0, 2, 1, 3).reshape(Bsz, Lq, N_HEAD * DV)
    return (ctxv @ Wo + bo).astype(np.float32)


def run_on_hw(in_maps, trace=False):
    from concourse.bass_utils import run_bass_kernel_spmd
    nc = _get_nc()
    return run_bass_kernel_spmd(nc, in_maps, list(range(N_CORES)), trace=trace)


def kernel(x, Wq, Wk, Wv, Wo, bo, mask, _trace=False, _results=None):
    x = np.asarray(x, dtype=np.float32)
    Wq = np.asarray(Wq, dtype=np.float32)
    Wk = np.asarray(Wk, dtype=np.float32)
    Wv = np.asarray(Wv, dtype=np.float32)
    Wo = np.asarray(Wo, dtype=np.float32)
    bo = np.asarray(bo, dtype=np.float32)
    mask_np = np.asarray(mask).reshape(mask.shape[-2], mask.shape[-1])

    causal = bool(np.array_equal(
        mask_np, np.tril(np.ones((L, L), dtype=bool))))
    if not causal or x.shape != (B, L, D):
        return _numpy_fallback(np.asarray(x), Wq, Wk, Wv, Wo, bo,
                               np.asarray(mask))

    res = run_on_hw(make_in_maps(x, Wq, Wk, Wv, Wo), trace=_trace)
    if _results is not None:
        _results.append(res)
    out = np.empty((B, L, D), dtype=np.float32)
    for b in range(B):
        out[b] = (np.asarray(res.results[2 * b]["out"], dtype=np.float32)
                  + np.asarray(res.results[2 * b + 1]["out"], dtype=np.float32)
                  + bo)
    return out

